# revision 1
# baseline (speedup 1.0000x reference)
"""Trainium2 Bass kernel for nn_NeuralTensorFactorization (8-core SPMD).

Strategy (validated against reference in golden.py):
- The TKAN scan depends on a sample only through t_id in [0,128): run the scan
  once over all 128 distinct tids (on every core, feature-major, fp32), then
  gather per-sample t_feat with a one-hot matmul.
- Scan step-s input for tid is embedding row 55000+tid+s: contiguous windows of
  a 135-row block, loaded once and PE-transposed; step inputs are free-dim
  slices (no per-step DMA).
- BatchNorm is shift-invariant => drop b0/b1 and center t_feat by its tid-mean
  (computed on device). This removes a catastrophic fp32 cancellation
  (raw y0 has |mu|~2.8 vs var ~1e-5).
- Batch (16384) sharded 2048/core. u/i embeddings gathered via indirect DMA and
  PE-transposed to feature-major. MLP runs feature-major; BN stats are (sum,
  sumsq) partial per core, combined with two tiny AllReduces (8KB / 2KB).
- fp32 everywhere (matmul dtype configurable; extra probe outputs measure
  f32r/bf16 matmul error on real data for tuning).
"""

import numpy as np

# ---------------- model constants (hardcoded; must match the problem) --------
NUM_TIMES, NUM_USERS, NUM_ITEMS = 128, 5000, 50000
STEP, D, B = 8, 256, 16384
MLP0, MLP1 = 1024, 256
NCORES = 8
BC = B // NCORES              # 2048 rows per core
NB = BC // 128                # 16 batch chunks of 128
TROW0 = 55000                 # first embedding row used by the scan windows
NTROWS = NUM_TIMES + STEP - 1 # 135 rows
EMB_ROWS = NUM_TIMES + NUM_USERS + NUM_ITEMS + STEP  # 55136

CFG = dict(
    mlp_dt="f32r",     # f32 | f32r  (heavy MLP matmuls)
    probes=True,
)

# ---------------- host-side spline conversion (fp64) -------------------------
SPLINE_ORDER, GRID_SIZE = 3, 5
_H = 2.0 / GRID_SIZE
_KNOTS = np.arange(-SPLINE_ORDER, GRID_SIZE + SPLINE_ORDER + 1, dtype=np.float64) * _H - 1.0


def _bspline_basis64(x):
    knots = _KNOTS
    xe = np.asarray(x, np.float64)[..., None]
    b = ((xe >= knots[:-1]) & (xe < knots[1:])).astype(np.float64)
    for d in range(1, SPLINE_ORDER + 1):
        left = (xe - knots[:-(d + 1)]) / (knots[d:-1] - knots[:-(d + 1)]) * b[..., :-1]
        right = (knots[d + 1:] - xe) / (knots[d + 1:] - knots[1:-d]) * b[..., 1:]
        b = left + right
    return b


def _spline_pp(spline_coef):
    """(d0..d3, a1..a4) for S(x)=d0+d1*u+d2*u^2+d3*u^3+sum a_k*relu(xm-b_k)^3,
    xm = min(x,1), u = max(xm,-1)+1, b = [-0.6,-0.2,0.2,0.6]."""
    coef = np.asarray(spline_coef, np.float64)
    breaks = np.array([-1.0, -0.6, -0.2, 0.2, 0.6, 1.0])
    polys = []
    for i in range(5):
        xs = np.linspace(breaks[i], breaks[i + 1], 6)[1:-1]
        ys = _bspline_basis64(xs) @ coef
        polys.append(np.polyfit(xs, ys, 3))
    p0 = np.poly1d(polys[0])
    d = np.array([p0(-1.0), p0.deriv(1)(-1.0), p0.deriv(2)(-1.0) / 2.0,
                  p0.deriv(3)(-1.0) / 6.0])
    a = np.array([polys[i][0] - polys[i - 1][0] for i in range(1, 5)])
    return np.concatenate([d, a]).astype(np.float32)  # (8,)


# ---------------- device program ---------------------------------------------

def _emit(nc, tc, ctx):
    import concourse.bass as bass
    import concourse.mybir as mybir
    from concourse.masks import make_identity
    dt = mybir.dt
    f32 = dt.float32
    ACT = mybir.ActivationFunctionType
    ALU = mybir.AluOpType
    X = mybir.AxisListType.X
    P = 128

    fmm = dt.float32r if CFG["mlp_dt"] == "f32r" else f32

    def mmcast(ap):
        return ap

    # ---- DRAM I/O ----
    def din(name, shape, dtype=f32):
        return nc.dram_tensor(name, shape, dtype, kind="ExternalInput").ap()

    emb = din("emb", (EMB_ROWS, D))
    onehot_d = din("onehot", (128, BC))
    uidx_d = din("uidx", (128, NB), dt.int32)
    iidx_d = din("iidx", (128, NB), dt.int32)
    wg_d = din("wg", (D, 3 * D)); ug_d = din("ug", (D, 3 * D)); bg_d = din("bg", (3 * D,))
    wx_d = din("wx", (2, D, D)); wh_d = din("wh", (2, D, D))
    mix_d = din("mix", (2, 2, D))
    aggw_d = din("aggw", (2 * D, D)); aggb_d = din("aggb", (D,))
    projw_d = din("projw", (D, D)); projb_d = din("projb", (D,))
    w0_d = din("w0", (3 * D, MLP0)); w1_d = din("w1", (MLP0, MLP1))
    fcw_d = din("fcw", (D, 1)); fcb_d = din("fcb", (1,))
    g0_d = din("gamma0", (MLP0,)); be0_d = din("beta0", (MLP0,))
    g1_d = din("gamma1", (MLP1,)); be1_d = din("beta1", (MLP1,))
    spco_d = din("spco", (8,))

    out_d = nc.dram_tensor("out", (BC,), f32, kind="ExternalOutput").ap()
    if CFG["probes"]:
        pr_f32_d = nc.dram_tensor("pr_f32", (128, 512), f32, kind="ExternalOutput").ap()
        pr_f32r_d = nc.dram_tensor("pr_f32r", (128, 512), f32, kind="ExternalOutput").ap()
        pr_bf16_d = nc.dram_tensor("pr_bf16", (128, 512), f32, kind="ExternalOutput").ap()

    ccw_in = nc.dram_tensor("ccw_in", (128, 1), f32).ap()
    ccw_out = nc.dram_tensor("ccw_out", (128, 1), f32, addr_space="Shared").ap()
    cc0_in = nc.dram_tensor("cc0_in", (128, 16), f32).ap()
    cc0_out = nc.dram_tensor("cc0_out", (128, 16), f32, addr_space="Shared").ap()
    cc1_in = nc.dram_tensor("cc1_in", (128, 4), f32).ap()
    cc1_out = nc.dram_tensor("cc1_out", (128, 4), f32, addr_space="Shared").ap()

    # ---- whole-kernel pools ----
    consts = ctx.enter_context(tc.tile_pool(name="consts", bufs=1))
    zpool = ctx.enter_context(tc.tile_pool(name="zpool", bufs=1))

    def load(pool, name, dram_ap, shape, dtype=f32):
        t = pool.tile(shape, dtype, name=name)
        nc.sync.dma_start(out=t, in_=dram_ap)
        return t

    w0_sb = consts.tile([P, 6, MLP0], fmm, name="w0_sb")
    w1_sb = consts.tile([P, 8, MLP1], fmm, name="w1_sb")
    fcw_sb = consts.tile([P, 2, 1], fmm, name="fcw_sb")
    with tc.tile_pool(name="wload", bufs=1) as wload:
        w0_f = load(wload, "w0_f", w0_d.rearrange("(k p) m -> p k m", p=P), [P, 6, MLP0])
        nc.vector.tensor_copy(out=w0_sb, in_=w0_f)
        w1_f = load(wload, "w1_f", w1_d.rearrange("(k p) m -> p k m", p=P), [P, 8, MLP1])
        nc.vector.tensor_copy(out=w1_sb, in_=w1_f)
        fcw_f = load(wload, "fcw_f", fcw_d.rearrange("(k p) m -> p k m", p=P), [P, 2, 1])
        nc.vector.tensor_copy(out=fcw_sb, in_=fcw_f)
    g0_sb = load(consts, "g0_sb", g0_d.rearrange("(t p) -> p t", p=P), [P, 8])
    be0_sb = load(consts, "be0_sb", be0_d.rearrange("(t p) -> p t", p=P), [P, 8])
    g1_sb = load(consts, "g1_sb", g1_d.rearrange("(t p) -> p t", p=P), [P, 2])
    be1_sb = load(consts, "be1_sb", be1_d.rearrange("(t p) -> p t", p=P), [P, 2])
    fcb_sb = load(consts, "fcb_sb", fcb_d.unsqueeze(0), [1, 1])
    uidx_sb = load(consts, "uidx_sb", uidx_d, [P, NB], dt.int32)
    iidx_sb = load(consts, "iidx_sb", iidx_d, [P, NB], dt.int32)
    ident = consts.tile([P, P], f32, name="ident")
    make_identity(nc, ident)
    # collective warmup: absorbs first-collective cold-start + core launch skew
    # concurrently with the compute prologue (runs on the CC engine).
    warm_sb = consts.tile([P, 1], f32, name="warm_sb")
    nc.vector.memset(warm_sb, 1.0)
    nc.sync.dma_start(out=ccw_in, in_=warm_sb)
    nc.gpsimd.collective_compute(
        "AllReduce", ALU.add, replica_groups=[list(range(NCORES))],
        ins=[ccw_in.opt()], outs=[ccw_out.opt()])

    zk = [zpool.tile([P, BC], fmm, name=f"zk{i}") for i in range(6)]

    # =====================================================================
    # Stages 1-3 in a scoped pool set (released before the MLP)
    # =====================================================================
    from contextlib import ExitStack
    with ExitStack() as sctx:
        pA = sctx.enter_context(tc.tile_pool(name="pA", bufs=1))
        stp = sctx.enter_context(tc.tile_pool(name="stp", bufs=2))
        ps_tr = sctx.enter_context(tc.tile_pool(name="ps_tr", bufs=2, space="PSUM"))
        scan_psum_scope = ExitStack()
        ps_scan = scan_psum_scope.enter_context(
            tc.tile_pool(name="ps_scan", bufs=1, space="PSUM"))

        wg_sb = load(pA, "wg_sb", wg_d.rearrange("(k p) m -> p k m", p=P), [P, 2, 3 * D])
        ug_sb = load(pA, "ug_sb", ug_d.rearrange("(k p) m -> p k m", p=P), [P, 2, 3 * D])
        wx_sb = load(pA, "wx_sb", wx_d.rearrange("s (k p) m -> p s k m", p=P), [P, 2, 2, D])
        wh_sb = load(pA, "wh_sb", wh_d.rearrange("s (k p) m -> p s k m", p=P), [P, 2, 2, D])
        aggw_sb = load(pA, "aggw_sb", aggw_d.rearrange("(k p) m -> p k m", p=P), [P, 4, D])
        projw_sb = load(pA, "projw_sb", projw_d.rearrange("(k p) m -> p k m", p=P), [P, 2, D])
        bg_sb = load(pA, "bg_sb", bg_d.rearrange("(t p) -> p t", p=P), [P, 6])
        aggb_sb = load(pA, "aggb_sb", aggb_d.rearrange("(t p) -> p t", p=P), [P, 2])
        mix_sb = load(pA, "mix_sb", mix_d.rearrange("s j (t p) -> p s j t", p=P), [P, 2, 2, 2])
        projb_bc = load(pA, "projb_bc", projb_d.unsqueeze(0).to_broadcast([P, D]), [P, D])
        spco = load(pA, "spco", spco_d.unsqueeze(0).to_broadcast([P, 8]), [P, 8])
        onehot_sb = load(pA, "onehot_sb", onehot_d, [P, BC])
        ones_sb = pA.tile([P, 1], f32, name="ones_sb")
        nc.vector.memset(ones_sb, 1.0)

        def sp(j):  # spline scalar j as per-partition AP
            return spco[:, j:j + 1]

        # ---- Stage 1: T block load + transpose -> T_T (128, 2, 136) ----
        T_T = pA.tile([P, 2, NTROWS + 1], f32, name="T_T")
        tb0 = pA.tile([P, D], f32, name="tb0")
        nc.sync.dma_start(out=tb0, in_=emb[TROW0:TROW0 + 128, :])
        tb1 = pA.tile([P, D], f32, name="tb1")
        nc.sync.dma_start(out=tb1, in_=emb[TROW0 + 7:TROW0 + NTROWS, :])
        for ft in range(2):
            pt = ps_tr.tile([P, P], f32, name="pt_a", tag="pt")
            nc.tensor.transpose(out=pt, in_=tb0[:, ft * P:(ft + 1) * P], identity=ident)
            nc.scalar.copy(out=T_T[:, ft, 0:128], in_=pt)
        for ft in range(2):
            pt = ps_tr.tile([P, P], f32, name="pt_b", tag="pt")
            nc.tensor.transpose(out=pt, in_=tb1[:, ft * P:(ft + 1) * P], identity=ident)
            nc.scalar.copy(out=T_T[:, ft, 128:135], in_=pt[:, 121:128])

        # ---- Stage 2: the scan (feature-major, 128 tids, fp32) ----
        h_T = pA.tile([P, 2, 128], f32, name="h_T")
        c_T = pA.tile([P, 2, 128], f32, name="c_T")
        sub_T = pA.tile([P, 2, 2, 128], f32, name="sub_T")

        TT = lambda kt, s: T_T[:, kt, s:s + 128]

        for s in range(STEP):
            first = s == 0
            # gates^T: psA holds m=0..3 (i,f), psB m=4,5 (g)
            psA = ps_scan.tile([P, 512], f32, name="psA", tag="psA", bufs=2)
            psB = ps_scan.tile([P, 256], f32, name="psB", tag="psB", bufs=1)
            for m in range(6):
                tgt = psA[:, (m % 4) * P:(m % 4 + 1) * P] if m < 4 else \
                      psB[:, (m - 4) * P:(m - 3) * P]
                msl = slice(m * P, (m + 1) * P)
                for kt in range(2):
                    nc.tensor.matmul(tgt, lhsT=wg_sb[:, kt, msl], rhs=TT(kt, s),
                                     start=(kt == 0), stop=(first and kt == 1))
                if not first:
                    for kt in range(2):
                        nc.tensor.matmul(tgt, lhsT=ug_sb[:, kt, msl],
                                         rhs=h_T[:, kt, :],
                                         start=False, stop=(kt == 1))
            ifg = stp.tile([P, 4, 128], f32, name="ifg", tag="ifg")
            for m in range(4):
                nc.scalar.activation(out=ifg[:, m, :], in_=psA[:, m * P:(m + 1) * P],
                                     func=ACT.Sigmoid, bias=bg_sb[:, m:m + 1])
            g_t = stp.tile([P, 2, 128], f32, name="g_t", tag="g_t")
            for j in range(2):
                nc.scalar.activation(out=g_t[:, j, :], in_=psB[:, j * P:(j + 1) * P],
                                     func=ACT.Tanh, bias=bg_sb[:, 4 + j:5 + j])

            # agg: (128, [sub, ft, tid])
            psC = ps_scan.tile([P, 2, 2, 128], f32, name="psC", tag="psC", bufs=2)
            for k in range(2):
                for m in range(2):
                    tgt = psC[:, k, m, :]
                    msl = slice(m * P, (m + 1) * P)
                    for kt in range(2):
                        nc.tensor.matmul(tgt, lhsT=wx_sb[:, k, kt, msl],
                                         rhs=TT(kt, s),
                                         start=(kt == 0), stop=(first and kt == 1))
                    if not first:
                        for kt in range(2):
                            nc.tensor.matmul(tgt, lhsT=wh_sb[:, k, kt, msl],
                                             rhs=sub_T[:, k, kt, :],
                                             start=False, stop=(kt == 1))

            # spline s0 = S(agg0) in truncated power form (DVE)
            agg0 = psC[:, 0]
            sq = lambda nm: stp.tile([P, 2, 128], f32, name=nm, tag=nm)
            xm = sq("xm")
            nc.vector.tensor_scalar(out=xm, in0=agg0, scalar1=1.0, scalar2=None,
                                    op0=ALU.min)
            u = sq("u")
            nc.vector.tensor_scalar(out=u, in0=xm, scalar1=-1.0, scalar2=1.0,
                                    op0=ALU.max, op1=ALU.add)
            q = sq("q")
            nc.vector.tensor_scalar(out=q, in0=u, scalar1=sp(3), scalar2=sp(2),
                                    op0=ALU.mult, op1=ALU.add)
            q2 = sq("q2")
            nc.vector.scalar_tensor_tensor(out=q2, in0=q, scalar=0.0, in1=u,
                                           op0=ALU.add, op1=ALU.mult)
            r1 = sq("r1")
            nc.vector.scalar_tensor_tensor(out=r1, in0=q2, scalar=sp(1), in1=u,
                                           op0=ALU.add, op1=ALU.mult)
            bks = (0.6, 0.2, -0.2, -0.6)
            ws = []
            for k in range(4):
                uk = sq(f"uk{k}")
                nc.vector.tensor_scalar(out=uk, in0=xm, scalar1=bks[k], scalar2=0.0,
                                        op0=ALU.add, op1=ALU.max)
                vk = sq(f"vk{k}")
                nc.vector.tensor_tensor(out=vk, in0=uk, in1=uk, op=ALU.mult)
                wk = sq(f"wk{k}")
                nc.vector.scalar_tensor_tensor(out=wk, in0=vk, scalar=sp(4 + k),
                                               in1=uk, op0=ALU.mult, op1=ALU.mult)
                ws.append(wk)
            p1 = sq("p1"); nc.vector.tensor_tensor(out=p1, in0=ws[0], in1=ws[1], op=ALU.add)
            p2 = sq("p2"); nc.vector.tensor_tensor(out=p2, in0=ws[2], in1=ws[3], op=ALU.add)
            p3 = sq("p3"); nc.vector.tensor_tensor(out=p3, in0=p1, in1=p2, op=ALU.add)
            s0 = sq("s0")
            nc.vector.scalar_tensor_tensor(out=s0, in0=r1, scalar=sp(0), in1=p3,
                                           op0=ALU.add, op1=ALU.add)
            s1 = sq("s1")
            nc.scalar.activation(out=s1, in_=psC[:, 1], func=ACT.Square)

            # new_sub = mixA*sub_out + mixB*sub
            souts = (s0, s1)
            for k in range(2):
                mixA = mix_sb[:, k, 0, :].unsqueeze(-1).to_broadcast([P, 2, 128])
                mixB = mix_sb[:, k, 1, :].unsqueeze(-1).to_broadcast([P, 2, 128])
                t2 = stp.tile([P, 2, 128], f32, name=f"t2_{k}", tag=f"t2_{k}")
                nc.vector.tensor_tensor(out=t2, in0=souts[k], in1=mixA, op=ALU.mult)
                if first:
                    nc.vector.tensor_copy(out=sub_T[:, k], in_=t2)
                else:
                    tmp = stp.tile([P, 2, 128], f32, name=f"tm_{k}", tag=f"tm_{k}")
                    nc.vector.tensor_tensor(out=tmp, in0=sub_T[:, k], in1=mixB,
                                            op=ALU.mult)
                    nc.vector.tensor_tensor(out=sub_T[:, k], in0=t2, in1=tmp,
                                            op=ALU.add)

            # o = sigmoid([s0;s1]@aggW + aggb)
            psD = ps_scan.tile([P, 256], f32, name="psD", tag="psD", bufs=1)
            scat = (s0[:, 0, :], s0[:, 1, :], s1[:, 0, :], s1[:, 1, :])
            for m in range(2):
                tgt = psD[:, m * P:(m + 1) * P]
                msl = slice(m * P, (m + 1) * P)
                for kt in range(4):
                    nc.tensor.matmul(tgt, lhsT=aggw_sb[:, kt, msl], rhs=scat[kt],
                                     start=(kt == 0), stop=(kt == 3))
            o_t = stp.tile([P, 2, 128], f32, name="o_t", tag="o_t")
            for m in range(2):
                nc.scalar.activation(out=o_t[:, m, :], in_=psD[:, m * P:(m + 1) * P],
                                     func=ACT.Sigmoid, bias=aggb_sb[:, m:m + 1])

            # c, h updates
            t1 = stp.tile([P, 2, 128], f32, name="t1c", tag="t1c")
            nc.vector.tensor_tensor(out=t1, in0=ifg[:, 0:2], in1=g_t, op=ALU.mult)
            if first:
                nc.vector.tensor_copy(out=c_T, in_=t1)
            else:
                t2c = stp.tile([P, 2, 128], f32, name="t2c", tag="t2c")
                nc.vector.tensor_tensor(out=t2c, in0=ifg[:, 2:4], in1=c_T, op=ALU.mult)
                nc.vector.tensor_tensor(out=c_T, in0=t1, in1=t2c, op=ALU.add)
            tnc = stp.tile([P, 2, 128], f32, name="tnc", tag="tnc")
            nc.scalar.activation(out=tnc, in_=c_T, func=ACT.Tanh)
            nc.vector.tensor_tensor(out=h_T, in0=o_t, in1=tnc, op=ALU.mult)

        # ---- Stage 3: t_feat, tid-mean, one-hot gather -> zk[4], zk[5] ----
        scan_psum_scope.close()
        ps_sm = sctx.enter_context(tc.tile_pool(name="ps_sm", bufs=1, space="PSUM"))
        psT = ps_sm.tile([P, D], f32, name="psT", tag="psT")
        for kt in range(2):
            nc.tensor.matmul(psT, lhsT=h_T[:, kt, :], rhs=projw_sb[:, kt, :],
                             start=(kt == 0), stop=(kt == 1))
        tf_pre = stp.tile([P, D], f32, name="tf_pre", tag="tf_pre")
        nc.vector.tensor_tensor(out=tf_pre, in0=psT, in1=projb_bc, op=ALU.add)
        tf = pA.tile([P, D], f32, name="tf")
        nc.scalar.activation(out=tf, in_=tf_pre, func=ACT.Sigmoid)

        tbar = pA.tile([P, 2], f32, name="tbar")
        for ft in range(2):
            psb = ps_sm.tile([P, 1], f32, name="psb", tag="psb")
            nc.tensor.matmul(psb, lhsT=tf[:, ft * P:(ft + 1) * P], rhs=ones_sb,
                             start=True, stop=True)
            nc.scalar.mul(out=tbar[:, ft:ft + 1], in_=psb, mul=1.0 / 128.0)

        for mf in range(2):
            for n in range(4):
                pst = ps_sm.tile([P, 512], f32, name="pst", tag="pst", bufs=2)
                nc.tensor.matmul(pst, lhsT=tf[:, mf * P:(mf + 1) * P],
                                 rhs=onehot_sb[:, n * 512:(n + 1) * 512],
                                 start=True, stop=True)
                nc.vector.tensor_scalar(out=zk[4 + mf][:, n * 512:(n + 1) * 512],
                                        in0=pst, scalar1=tbar[:, mf:mf + 1],
                                        scalar2=None, op0=ALU.subtract)

        # ---- Stage 4: u/i gathers + transposes -> zk[0..3] ----
        gat = sctx.enter_context(tc.tile_pool(name="gat", bufs=3))
        for src, idx_sb in ((0, uidx_sb), (1, iidx_sb)):
            for c in range(NB):
                gt = gat.tile([P, D], f32, name=f"gt{src}", tag=f"gt{src}")
                nc.gpsimd.indirect_dma_start(
                    out=gt, out_offset=None, in_=emb,
                    in_offset=bass.IndirectOffsetOnAxis(ap=idx_sb[:, c:c + 1], axis=0))
                for ft in range(2):
                    pt = ps_tr.tile([P, P], f32, name="pt_g", tag="pt")
                    nc.tensor.transpose(out=pt, in_=gt[:, ft * P:(ft + 1) * P],
                                        identity=ident)
                    if (c + ft) % 2 == 0:
                        nc.scalar.copy(out=zk[2 * src + ft][:, c * P:(c + 1) * P], in_=pt)
                    else:
                        nc.vector.tensor_copy(out=zk[2 * src + ft][:, c * P:(c + 1) * P],
                                              in_=pt)

    # =====================================================================
    # Stage 5: MLP0 (y0^T = W0^T @ z'), stats, AR0, bn+relu
    # =====================================================================
    ypool = ctx.enter_context(tc.tile_pool(name="ypool", bufs=1))
    spool = ctx.enter_context(tc.tile_pool(name="spool", bufs=1))
    st3 = ctx.enter_context(tc.tile_pool(name="st3", bufs=1))
    scr = ctx.enter_context(tc.tile_pool(name="scr", bufs=2))
    ps_mm = ctx.enter_context(tc.tile_pool(name="ps_mm", bufs=4, space="PSUM"))
    ps_fc = ctx.enter_context(tc.tile_pool(name="ps_fc", bufs=2, space="PSUM"))

    y0 = [ypool.tile([P, BC], fmm, name=f"y0_{m}") for m in range(8)]
    ssum0 = spool.tile([P, 8, 4], f32, name="ssum0")
    sssq0 = spool.tile([P, 8, 4], f32, name="sssq0")
    for m in range(8):
        msl = slice(m * P, (m + 1) * P)
        for n in range(4):
            nsl = slice(n * 512, (n + 1) * 512)
            ps = ps_mm.tile([P, 512], f32, name="ps0", tag="mm")
            korder = (4, 5, 0, 1, 2, 3)
            for i, kt in enumerate(korder):
                nc.tensor.matmul(ps, lhsT=mmcast(w0_sb[:, kt, msl]),
                                 rhs=mmcast(zk[kt][:, nsl]),
                                 start=(i == 0), stop=(i == 5))
            if m % 2 == 0:
                nc.scalar.activation(out=y0[m][:, nsl], in_=ps, func=ACT.Identity,
                                     accum_out=ssum0[:, m, n:n + 1])
            else:
                nc.vector.tensor_scalar(out=y0[m][:, nsl], in0=ps, scalar1=0.0,
                                        scalar2=0.0, op0=ALU.add, op1=ALU.add,
                                        accum_out=ssum0[:, m, n:n + 1])
            sc = scr.tile([P, 512], f32, name="sc0", tag="sc")
            nc.vector.scalar_tensor_tensor(out=sc, in0=y0[m][:, nsl], scalar=0.0,
                                           in1=y0[m][:, nsl],
                                           op0=ALU.add, op1=ALU.mult,
                                           accum_out=sssq0[:, m, n:n + 1])

    stats0 = spool.tile([P, 16], f32, name="stats0")
    for m in range(8):
        nc.vector.tensor_reduce(out=stats0[:, m:m + 1], in_=ssum0[:, m, :],
                                axis=X, op=ALU.add)
        nc.vector.tensor_reduce(out=stats0[:, 8 + m:9 + m], in_=sssq0[:, m, :],
                                axis=X, op=ALU.add)
    nc.sync.dma_start(out=cc0_in, in_=stats0)
    nc.gpsimd.collective_compute(
        "AllReduce", ALU.add, replica_groups=[list(range(NCORES))],
        ins=[cc0_in.opt()], outs=[cc0_out.opt()])
    gstats0 = spool.tile([P, 16], f32, name="gstats0")
    nc.sync.dma_start(out=gstats0, in_=cc0_out)

    def bn_coefs(gstats, nm, gamma_sb, beta_sb, width):
        def t(name):
            return spool.tile([P, width], f32, name=f"{name}{nm}")
        mu = t("mu")
        nc.vector.tensor_scalar(out=mu, in0=gstats[:, 0:width], scalar1=1.0 / B,
                                scalar2=None, op0=ALU.mult)
        ey2 = t("ey2")
        nc.vector.tensor_scalar(out=ey2, in0=gstats[:, width:2 * width],
                                scalar1=1.0 / B, scalar2=None, op0=ALU.mult)
        var = t("var")
        nc.vector.scalar_tensor_tensor(out=var, in0=mu, scalar=0.0, in1=mu,
                                       op0=ALU.add, op1=ALU.mult)
        nc.vector.tensor_tensor(out=var, in0=ey2, in1=var, op=ALU.subtract)
        vpe = t("vpe")
        nc.vector.tensor_scalar(out=vpe, in0=var, scalar1=1e-5, scalar2=None,
                                op0=ALU.add)
        sd = t("sd")
        nc.scalar.activation(out=sd, in_=vpe, func=ACT.Sqrt)
        # one Newton step: sd' = 0.5*(sd + (var+eps)/sd), then rstd = 1/sd'
        rc = t("rc")
        nc.vector.reciprocal(out=rc, in_=sd)
        tn = t("tn")
        nc.vector.tensor_tensor(out=tn, in0=vpe, in1=rc, op=ALU.mult)
        nc.vector.tensor_tensor(out=tn, in0=tn, in1=sd, op=ALU.add)
        sd2 = t("sd2")
        nc.vector.tensor_scalar(out=sd2, in0=tn, scalar1=0.5, scalar2=None,
                                op0=ALU.mult)
        rstd = t("rstd")
        nc.vector.reciprocal(out=rstd, in_=sd2)
        scale = t("scale")
        nc.vector.tensor_tensor(out=scale, in0=gamma_sb, in1=rstd, op=ALU.mult)
        shift = t("shift")
        nc.vector.tensor_tensor(out=shift, in0=mu, in1=scale, op=ALU.mult)
        nc.vector.tensor_tensor(out=shift, in0=beta_sb, in1=shift, op=ALU.subtract)
        return scale, shift

    scale0, shift0 = bn_coefs(gstats0, "0", g0_sb, be0_sb, 8)
    for m in range(8):
        if m % 2 == 0:
            nc.scalar.activation(out=y0[m], in_=y0[m], func=ACT.Relu,
                                 bias=shift0[:, m:m + 1], scale=scale0[:, m:m + 1])
        else:
            nc.vector.tensor_scalar(out=y0[m], in0=y0[m],
                                    scalar1=scale0[:, m:m + 1],
                                    scalar2=shift0[:, m:m + 1],
                                    op0=ALU.mult, op1=ALU.add)
            nc.vector.tensor_scalar(out=y0[m], in0=y0[m], scalar1=0.0,
                                    scalar2=None, op0=ALU.max)

    # =====================================================================
    # Stage 6: MLP1, stats, AR1, bn+relu, fc, output
    # =====================================================================
    # reuse two dead z tiles as y1 storage (zk reads all precede y1 writes;
    # zk[0]/zk[1] stay untouched for the probes)
    y1 = [zk[2], zk[3]]
    ssum1 = spool.tile([P, 2, 4], f32, name="ssum1")
    sssq1 = spool.tile([P, 2, 4], f32, name="sssq1")
    for m in range(2):
        msl = slice(m * P, (m + 1) * P)
        for n in range(4):
            nsl = slice(n * 512, (n + 1) * 512)
            ps = ps_mm.tile([P, 512], f32, name="ps1", tag="mm")
            for kt in range(8):
                nc.tensor.matmul(ps, lhsT=mmcast(w1_sb[:, kt, msl]),
                                 rhs=mmcast(y0[kt][:, nsl]),
                                 start=(kt == 0), stop=(kt == 7))
            nc.scalar.activation(out=y1[m][:, nsl], in_=ps, func=ACT.Identity,
                                 accum_out=ssum1[:, m, n:n + 1])
            sc = scr.tile([P, 512], f32, name="sc1", tag="sc")
            nc.vector.scalar_tensor_tensor(out=sc, in0=y1[m][:, nsl], scalar=0.0,
                                           in1=y1[m][:, nsl],
                                           op0=ALU.add, op1=ALU.mult,
                                           accum_out=sssq1[:, m, n:n + 1])
    stats1 = spool.tile([P, 4], f32, name="stats1")
    for m in range(2):
        nc.vector.tensor_reduce(out=stats1[:, m:m + 1], in_=ssum1[:, m, :],
                                axis=X, op=ALU.add)
        nc.vector.tensor_reduce(out=stats1[:, 2 + m:3 + m], in_=sssq1[:, m, :],
                                axis=X, op=ALU.add)
    nc.sync.dma_start(out=cc1_in, in_=stats1)
    nc.gpsimd.collective_compute(
        "AllReduce", ALU.add, replica_groups=[list(range(NCORES))],
        ins=[cc1_in.opt()], outs=[cc1_out.opt()])
    gstats1 = spool.tile([P, 4], f32, name="gstats1")
    nc.sync.dma_start(out=gstats1, in_=cc1_out)

    scale1, shift1 = bn_coefs(gstats1, "1", g1_sb, be1_sb, 2)
    for m in range(2):
        nc.scalar.activation(out=y1[m], in_=y1[m], func=ACT.Relu,
                             bias=shift1[:, m:m + 1], scale=scale1[:, m:m + 1])

    out_sb = st3.tile([1, BC], f32, name="out_sb")
    for n in range(4):
        nsl = slice(n * 512, (n + 1) * 512)
        psf = ps_fc.tile([1, 512], f32, name="psf", tag="psf")
        for kt in range(2):
            nc.tensor.matmul(psf, lhsT=fcw_sb[:, kt, :], rhs=y1[kt][:, nsl],
                             start=(kt == 0), stop=(kt == 1))
        nc.vector.tensor_scalar(out=out_sb[:, nsl], in0=psf,
                                scalar1=fcb_sb[0:1, 0:1], scalar2=None, op0=ALU.add)
    nc.sync.dma_start(out=out_d, in_=out_sb)

    # ---- probes: same matmul in f32 / f32r / bf16 for error calibration ----
    if CFG["probes"]:
        dtb = dt.bfloat16
        A = w0_sb[:, 0, 0:128]
        Rr = zk[0][:, 0:512]
        pp = ps_mm.tile([P, 512], f32, name="pp", tag="mm")
        nc.tensor.matmul(pp, lhsT=A, rhs=Rr, start=True, stop=True)
        pr = st3.tile([P, 512], f32, name="pr_f32_sb")
        nc.scalar.copy(out=pr, in_=pp)
        nc.sync.dma_start(out=pr_f32_d, in_=pr)
        ar = st3.tile([P, 128], dt.float32r, name="ar")
        nc.vector.tensor_copy(out=ar, in_=A)
        rr = st3.tile([P, 512], dt.float32r, name="rr")
        nc.vector.tensor_copy(out=rr, in_=Rr)
        pp2 = ps_mm.tile([P, 512], f32, name="pp2", tag="mm")
        nc.tensor.matmul(pp2, lhsT=ar, rhs=rr, start=True, stop=True)
        pr2 = st3.tile([P, 512], f32, name="pr_f32r_sb")
        nc.scalar.copy(out=pr2, in_=pp2)
        nc.sync.dma_start(out=pr_f32r_d, in_=pr2)
        ab = st3.tile([P, 128], dtb, name="ab")
        nc.vector.tensor_copy(out=ab, in_=A)
        rb = st3.tile([P, 512], dtb, name="rb")
        nc.vector.tensor_copy(out=rb, in_=Rr)
        pp3 = ps_mm.tile([P, 512], f32, name="pp3", tag="mm")
        nc.tensor.matmul(pp3, lhsT=ab, rhs=rb, start=True, stop=True)
        pr3 = st3.tile([P, 512], f32, name="pr_bf16_sb")
        nc.scalar.copy(out=pr3, in_=pp3)
        nc.sync.dma_start(out=pr_bf16_d, in_=pr3)


# ---------------- module build + run -----------------------------------------
_CACHE = {}


def build_module():
    from contextlib import ExitStack
    import concourse.bacc as bacc
    import concourse.tile as tile
    nc = bacc.Bacc("TRN2", target_bir_lowering=False, debug=False,
                   num_devices=NCORES)
    with tile.TileContext(nc) as tc:
        with ExitStack() as ctx:
            _emit(nc, tc, ctx)
    nc.compile()
    return nc


def _get_module():
    if "nc" not in _CACHE:
        _CACHE["nc"] = build_module()
    return _CACHE["nc"]


def host_prep(inputs):
    """Build per-core input maps from the full input dict."""
    gi = {k: np.asarray(v) for k, v in inputs.items()}
    x = gi["x"].astype(np.int64)
    t_id = x[:, 0]
    u_id = np.clip(x[:, 1], 0, EMB_ROWS - 1).astype(np.int32)
    i_id = np.clip(x[:, 2] + NUM_USERS, 0, EMB_ROWS - 1).astype(np.int32)
    spco = _spline_pp(gi["spline_coef"])

    shared = dict(
        emb=gi["embedding"], wg=gi["Wg"], ug=gi["Ug"], bg=gi["bg"],
        wx=gi["sub_Wx"], wh=gi["sub_Wh"], mix=gi["sub_mix"],
        aggw=gi["aggW"], aggb=gi["aggb"], projw=gi["projW"], projb=gi["projb"],
        w0=gi["W0"], w1=gi["W1"], fcw=gi["fcW"], fcb=gi["fcb"],
        gamma0=gi["gamma0"], beta0=gi["beta0"],
        gamma1=gi["gamma1"], beta1=gi["beta1"], spco=spco,
    )
    shared = {k: np.ascontiguousarray(np.asarray(v, np.float32))
              for k, v in shared.items()}

    in_maps = []
    for c in range(NCORES):
        rows = slice(c * BC, (c + 1) * BC)
        tid_c = t_id[rows]
        onehot = np.zeros((128, BC), np.float32)
        onehot[tid_c, np.arange(BC)] = 1.0
        m = dict(shared)
        m["onehot"] = onehot
        m["uidx"] = np.ascontiguousarray(u_id[rows].reshape(NB, 128).T)
        m["iidx"] = np.ascontiguousarray(i_id[rows].reshape(NB, 128).T)
        in_maps.append(m)
    return in_maps


def kernel(**inputs):
    from concourse import bass_utils
    nc = _get_module()
    in_maps = host_prep(inputs)
    res = bass_utils.run_bass_kernel_spmd(
        nc, in_maps, core_ids=list(range(NCORES)))
    _CACHE["last_results"] = res
    out = np.concatenate([res.results[c]["out"] for c in range(NCORES)])
    return out.astype(np.float32)



# revision 22
# speedup vs baseline: 1.8974x; 1.8974x over previous
"""Trainium2 Bass kernel for nn_NeuralTensorFactorization (8-core SPMD), v2.

Design (validated numerically in proto.py, rel err ~5.5e-3 vs fp64, tol 2e-2):
- Scan over the 128 distinct tids (batch-independent), feature-major, bf16
  matmuls (4x over f32), fp32 psum/c-state. x-dependent parts of the cell
  (x@Wg+bg, x@Wx) are host-folded into G (128,6,135) / A (128,4,135) tables
  (linear weight folding over the 135 embedding time rows); the device scan
  keeps the recurrence: Ug*h, Wh*sub, spline, mix, aggW, LSTM cell.
- Spline -> degree-6 poly (host minimax fit), evaluated with custom fused DVE
  ops (CLAMPSUM chain) ~5 DVE ops vs ~21 stock.
- Batch 16384 sharded 2048/core; u/i embeddings gathered from a bf16 copy of
  the table, PE-transposed to feature-major bf16 zk tiles.
- MLP0 via 5 matmuls/chunk: 4 emb chunks + V@onehot where V = tfc^T @ W0[512:]
  (associativity; replaces separate one-hot gather of t_feat).
- BN: per-feature (=partition) sum/sumsq accumulated on psum->sbuf copy (Act)
  and square (DVE), two tiny AllReduces; t_feat centered by tid-mean to avoid
  fp32 variance cancellation. bf16 y0/y1 storage.
"""

import numpy as np
import ml_dtypes

# ---------------- model constants (hardcoded; must match the problem) --------
NUM_TIMES, NUM_USERS, NUM_ITEMS = 128, 5000, 50000
STEP, D, B = 8, 256, 16384
MLP0, MLP1 = 1024, 256
NCORES = 8
BC = B // NCORES              # 2048 rows per core
NB = BC // 128                # 16 batch chunks of 128
TROW0 = 55000                 # first embedding row used by the scan windows
NTROWS = NUM_TIMES + STEP - 1 # 135 rows
EMB_ROWS = NUM_TIMES + NUM_USERS + NUM_ITEMS + STEP  # 55136

CFG = dict(
    probes=False,
    poly_deg=6,
)

BF16 = ml_dtypes.bfloat16

# ---------------- host-side spline poly fit (fp64) ---------------------------
SPLINE_ORDER, GRID_SIZE = 3, 5
_H = 2.0 / GRID_SIZE
_KNOTS = np.arange(-SPLINE_ORDER, GRID_SIZE + SPLINE_ORDER + 1,
                   dtype=np.float64) * _H - 1.0


def _bspline_basis64(x):
    knots = _KNOTS
    xe = np.asarray(x, np.float64)[..., None]
    b = ((xe >= knots[:-1]) & (xe < knots[1:])).astype(np.float64)
    for d in range(1, SPLINE_ORDER + 1):
        left = (xe - knots[:-(d + 1)]) / (knots[d:-1] - knots[:-(d + 1)]) * b[..., :-1]
        right = (knots[d + 1:] - xe) / (knots[d + 1:] - knots[1:-d]) * b[..., 1:]
        b = left + right
    return b


def _fit_poly(coef, deg):
    """Weighted LS fit of the clamped spline on [-1,1]; returns c[k] = coeff
    of t^k, ascending, float32, length deg+1."""
    xs = np.linspace(-1.0, 1.0, 4001)
    ys = _bspline_basis64(xs) @ np.asarray(coef, np.float64)
    w = np.ones_like(xs)
    for b in (-1.0, -0.6, -0.2, 0.2, 0.6, 1.0):
        w += 2.0 * np.exp(-((xs - b) / 0.05) ** 2)
    V = np.vander(xs, deg + 1)            # highest power first
    c = np.linalg.lstsq(V * w[:, None], ys * w, rcond=None)[0]
    return c[::-1].astype(np.float32)     # ascending


# ---------------- custom DVE ops ---------------------------------------------
_DVE_OPS = {}


def _register_dve_ops():
    """Idempotently register the fused DVE ops this kernel uses."""
    if _DVE_OPS:
        return _DVE_OPS
    from concourse import dve_ops as DOPS
    from concourse.dve_spec import (Spec, Src0, Src1, C0, C1, One, Zero,
                                    maxx, minn, sq, lower)
    from concourse.dve_uop import DveOpSpec

    def reg(name, spec):
        if name in DOPS._SUB_OPCODE_FOR_NAME:
            return next(o for o in DOPS.OPS if o.name == name)
        row = DOPS._CUSTOM_DVE_ROW_BASE + len(DOPS.OPS)
        shas = {}
        for v in ("v3", "v4"):
            uops = lower(spec, ver=v)
            shas[v] = DveOpSpec(name=name, opcode=row, uops=uops,
                                rd1_en=DOPS.has_src1(spec)).sha(v)
        op = DOPS.DveOp(name, spec, subdim=False, uops_sha=shas)
        DOPS._SUB_OPCODE_FOR_NAME[name] = row
        DOPS.OPS.append(op)
        DOPS.CUSTOM_DVE_SPECS[name] = spec
        return op

    def _clip(in0, imm2):
        return np.minimum(np.maximum(in0.astype(np.float32), imm2), 1.0)

    # t = clip(in0 + in1, -1, 1)
    _DVE_OPS["CLAMPSUM"] = reg("NTF_CLAMPSUM", Spec(
        body=minn(maxx(Src0 + Src1, Zero - One), One),
        reference=lambda in0, in1, s0, s1, imm2:
            _clip(in0 + in1, -1.0)))
    # out = (in0 + in1)^2
    _DVE_OPS["SUMSQ"] = reg("NTF_SUMSQ", Spec(
        body=sq(Src0 + Src1),
        reference=lambda in0, in1, s0, s1, imm2:
            ((in0.astype(np.float32) + in1) ** 2)))
    # Horner step: out = (in1*in0 + s0)*in0 + s1   (in0 = t, in1 = chain)
    _DVE_OPS["HORN2"] = reg("NTF_HORN2", Spec(
        body=(Src1 * Src0 + C0) * Src0 + C1,
        reference=lambda in0, in1, s0, s1, imm2:
            ((in1 * in0.astype(np.float32) + s0) * in0 + s1)))
    # out = in1*in0 + s0
    _DVE_OPS["HORN1"] = reg("NTF_HORN1", Spec(
        body=Src1 * Src0 + C0,
        reference=lambda in0, in1, s0, s1, imm2:
            (in1 * in0.astype(np.float32) + s0)))
    # out = in0*s0 + in1*s1
    _DVE_OPS["AXPBY"] = reg("NTF_AXPBY", Spec(
        body=Src0 * C0 + Src1 * C1,
        reference=lambda in0, in1, s0, s1, imm2:
            (in0.astype(np.float32) * s0 + in1 * s1)))
    return _DVE_OPS


# ---------------- device program ---------------------------------------------

def _emit(nc, tc, ctx):
    import concourse.bass as bass
    import concourse.mybir as mybir
    from concourse.masks import make_identity
    from contextlib import ExitStack
    OPS = _register_dve_ops()
    dt = mybir.dt
    f32 = dt.float32
    bf = dt.bfloat16
    ACT = mybir.ActivationFunctionType
    ALU = mybir.AluOpType
    P = 128

    # ---- DRAM I/O ----
    def din(name, shape, dtype=f32):
        return nc.dram_tensor(name, shape, dtype, kind="ExternalInput").ap()

    emb16 = din("emb16", (EMB_ROWS, D), bf)
    onehot_d = din("onehot", (P, BC), bf)
    uidx_d = din("uidx", (P, NB), dt.int32)
    iidx_d = din("iidx", (P, NB), dt.int32)
    g_d = din("gpre", (P, 6, NTROWS), bf)        # host-folded Wg^T.x + bg
    a_d = din("apre", (P, 4, NTROWS), bf)        # host-folded Wx^T.x
    ug_d = din("ug", (P, 2, 3 * D), bf)
    wh_d = din("wh", (P, 2, 2, D), bf)
    aggw_d = din("aggw", (P, 4, D), bf)
    projw_d = din("projw", (P, 2, D), bf)
    w0_d = din("w0", (P, 6, MLP0), bf)
    w1_d = din("w1", (P, 8, MLP1), bf)
    fcw_d = din("fcw", (P, 2, 1), bf)
    aggb_d = din("aggb", (P, 2))
    projb_d = din("projbrow", (1, D), bf)
    mix_d = din("mixsb", (P, 2, 2, 2))
    spco_d = din("spco", (1, 8))
    g0_d = din("gamma0", (P, 8)); be0_d = din("beta0", (P, 8))
    g1_d = din("gamma1", (P, 2)); be1_d = din("beta1", (P, 2))
    fcb_d = din("fcb", (1, 1))

    out_d = nc.dram_tensor("out", (BC,), f32, kind="ExternalOutput").ap()

    ccw_in = nc.dram_tensor("ccw_in", (P, 1), f32).ap()
    ccw_out = nc.dram_tensor("ccw_out", (P, 1), f32, addr_space="Shared").ap()
    cc0_in = nc.dram_tensor("cc0_in", (P, 16), f32).ap()
    cc0_out = nc.dram_tensor("cc0_out", (P, 16), f32, addr_space="Shared").ap()
    cc1_in = nc.dram_tensor("cc1_in", (P, 4), f32).ap()
    cc1_out = nc.dram_tensor("cc1_out", (P, 4), f32, addr_space="Shared").ap()

    # ---- whole-kernel pools ----
    consts = ctx.enter_context(tc.tile_pool(name="consts", bufs=1))
    zpool = ctx.enter_context(tc.tile_pool(name="zpool", bufs=1))

    def load(pool, name, dram_ap, shape, dtype=f32):
        t = pool.tile(shape, dtype, name=name)
        nc.sync.dma_start(out=t, in_=dram_ap)
        return t

    # scan-critical weights first (small; scan can start ~immediately)
    g_sb = load(consts, "g_sb", g_d, [P, 6, NTROWS], bf)
    a_sb = load(consts, "a_sb", a_d, [P, 4, NTROWS], bf)
    ug_sb = load(consts, "ug_sb", ug_d, [P, 2, 3 * D], bf)
    wh_sb = load(consts, "wh_sb", wh_d, [P, 2, 2, D], bf)
    aggw_sb = load(consts, "aggw_sb", aggw_d, [P, 4, D], bf)
    aggb_sb = load(consts, "aggb_sb", aggb_d, [P, 2])
    mix_sb = load(consts, "mix_sb", mix_d, [P, 2, 2, 2])
    spco = load(consts, "spco", spco_d.to_broadcast([P, 8]), [P, 8])
    projw_sb = load(consts, "projw_sb", projw_d, [P, 2, D], bf)
    projb_row = load(consts, "projb_row", projb_d, [1, D], bf)
    uidx_sb = load(consts, "uidx_sb", uidx_d, [P, NB], dt.int32)
    iidx_sb = load(consts, "iidx_sb", iidx_d, [P, NB], dt.int32)
    # big weights stream while the scan runs
    w0_sb = load(consts, "w0_sb", w0_d, [P, 6, MLP0], bf)
    onehot_sb = load(consts, "onehot_sb", onehot_d, [P, BC], bf)
    w1_sb = load(consts, "w1_sb", w1_d, [P, 8, MLP1], bf)
    fcw_sb = load(consts, "fcw_sb", fcw_d, [P, 2, 1], bf)
    g0_sb = load(consts, "g0_sb", g0_d, [P, 8])
    be0_sb = load(consts, "be0_sb", be0_d, [P, 8])
    g1_sb = load(consts, "g1_sb", g1_d, [P, 2])
    be1_sb = load(consts, "be1_sb", be1_d, [P, 2])
    fcb_sb = load(consts, "fcb_sb", fcb_d, [1, 1])

    ident16 = consts.tile([P, P], bf, name="ident16")
    make_identity(nc, ident16)
    ident32 = consts.tile([P, P], f32, name="ident32")
    make_identity(nc, ident32)
    ones_col = consts.tile([P, 1], f32, name="ones_col")
    nc.vector.memset(ones_col, 1.0)
    ones_row = consts.tile([1, P], bf, name="ones_row")
    nc.vector.memset(ones_row, 1.0)

    # collective warmup (absorbs first-collective cold-start)
    warm_sb = consts.tile([P, 1], f32, name="warm_sb")
    nc.vector.memset(warm_sb, 1.0)
    nc.sync.dma_start(out=ccw_in, in_=warm_sb)
    nc.gpsimd.collective_compute(
        "AllReduce", ALU.add, replica_groups=[list(range(NCORES))],
        ins=[ccw_in.opt()], outs=[ccw_out.opt()])

    def sp(j):  # poly coefficient j as per-partition scalar AP
        return spco[:, j:j + 1]

    zk = [zpool.tile([P, BC], bf, name=f"zk{i}") for i in range(4)]
    y0 = [zpool.tile([P, BC], bf, name=f"y0_{m}") for m in range(8)]
    V_sb = zpool.tile([P, MLP0], bf, name="V_sb")

    spool = ctx.enter_context(tc.tile_pool(name="spool", bufs=1))
    stats0 = spool.tile([P, 16], f32, name="stats0")
    stats1 = spool.tile([P, 4], f32, name="stats1")

    # =====================================================================
    # Scan + gathers (scoped pools released before the MLP)
    # =====================================================================
    with ExitStack() as sctx:
        pA = sctx.enter_context(tc.tile_pool(name="pA", bufs=1))
        stp = sctx.enter_context(tc.tile_pool(name="stp", bufs=2))
        ps_tr = sctx.enter_context(tc.tile_pool(name="ps_tr", bufs=2, space="PSUM"))
        scan_ps_scope = ExitStack()
        ps_scan = scan_ps_scope.enter_context(
            tc.tile_pool(name="ps_scan", bufs=1, space="PSUM"))

        # persistent scan state, feature-major
        h_T = pA.tile([P, 2, P], bf, name="h_T")
        c_T = pA.tile([P, 2, P], f32, name="c_T")
        sub_T = pA.tile([P, 2, 2, P], bf, name="sub_T")

        # ---- gather u/i embedding rows; 4-chunk blocks -> transpose -> zk --
        gt = {}
        for src, idx_sb in ((0, uidx_sb), (1, iidx_sb)):
            for c in range(NB):
                t = pA.tile([P, D], bf, name=f"gt{src}_{c}")
                nc.gpsimd.indirect_dma_start(
                    out=t, out_offset=None, in_=emb16,
                    in_offset=bass.IndirectOffsetOnAxis(
                        ap=idx_sb[:, c:c + 1], axis=0))
                gt[(src, c)] = t

        def gather_block(src, cb):
            # 4 chunks x 2 ft -> two [P,512] psum banks -> 2 copies into zk
            for ft in range(2):
                pt = ps_tr.tile([P, 512], bf, name="pt", tag="pt")
                for j in range(4):
                    c = cb * 4 + j
                    nc.tensor.transpose(out=pt[:, j * P:(j + 1) * P],
                                        in_=gt[(src, c)][:, ft * P:(ft + 1) * P],
                                        identity=ident16)
                dst = zk[2 * src + ft][:, cb * 512:(cb + 1) * 512]
                if (src + ft + cb) % 2 == 0:
                    nc.scalar.copy(out=dst, in_=pt)
                else:
                    nc.vector.tensor_copy(out=dst, in_=pt)

        gather_blocks = [(s, cb) for s in range(2) for cb in range(4)]

        # ---- the scan ----
        GS = lambda m0, m1, s: g_sb[:, m0:m1, s:s + P]
        AS = lambda k0, k1, s: a_sb[:, k0:k1, s:s + P]

        for s in range(STEP):
            first = s == 0
            if not first:
                psA = ps_scan.tile([P, 4, P], f32, name="psA", tag="psA", bufs=2)
                psB = ps_scan.tile([P, 2, P], f32, name="psB", tag="psB", bufs=1)
                for m in range(6):
                    tgt = psA[:, m, :] if m < 4 else psB[:, m - 4, :]
                    for kt in range(2):
                        nc.tensor.matmul(tgt, lhsT=ug_sb[:, kt, m * P:(m + 1) * P],
                                         rhs=h_T[:, kt, :],
                                         start=(kt == 0), stop=(kt == 1))
                # += host-folded gates_x (+bg)
                nc.vector.tensor_tensor(out=psA, in0=psA, in1=GS(0, 4, s),
                                        op=ALU.add)
                nc.vector.tensor_tensor(out=psB, in0=psB, in1=GS(4, 6, s),
                                        op=ALU.add)
                ifg = stp.tile([P, 4, P], bf, name="ifg", tag="ifg")
                nc.scalar.activation(out=ifg, in_=psA, func=ACT.Sigmoid)
                g_t = stp.tile([P, 2, P], bf, name="g_t", tag="g_t")
                nc.scalar.activation(out=g_t, in_=psB, func=ACT.Tanh)

                psC = ps_scan.tile([P, 2, 2, P], f32, name="psC", tag="psC",
                                   bufs=2)
                for k in range(2):
                    for mf in range(2):
                        tgt = psC[:, k, mf, :]
                        for kt in range(2):
                            nc.tensor.matmul(
                                tgt, lhsT=wh_sb[:, k, kt, mf * P:(mf + 1) * P],
                                rhs=sub_T[:, k, kt, :],
                                start=(kt == 0), stop=(kt == 1))
                agg0, agg1 = psC[:, 0], psC[:, 1]
                a0x, a1x = AS(0, 2, s), AS(2, 4, s)
            else:
                ifg = stp.tile([P, 4, P], bf, name="ifg", tag="ifg")
                nc.scalar.activation(out=ifg, in_=GS(0, 4, s), func=ACT.Sigmoid)
                g_t = stp.tile([P, 2, P], bf, name="g_t", tag="g_t")
                nc.scalar.activation(out=g_t, in_=GS(4, 6, s), func=ACT.Tanh)
                agg0, agg1 = None, None
                a0x, a1x = AS(0, 2, s), AS(2, 4, s)

            sq_ = lambda nm: stp.tile([P, 256], bf, name=nm, tag=nm)
            # spline input t = clip(agg0, -1, 1); s1 = agg1^2
            tcl = sq_("tcl")
            s1t = sq_("s1t")
            if first:
                nc.vector.tensor_scalar(out=tcl, in0=a0x, scalar1=1.0,
                                        scalar2=-1.0, op0=ALU.min, op1=ALU.max)
                nc.scalar.activation(out=s1t, in_=a1x, func=ACT.Square)
            else:
                nc.vector._custom_dve(OPS["CLAMPSUM"], out=tcl, in0=agg0,
                                      in1=a0x)
                nc.vector._custom_dve(OPS["SUMSQ"], out=s1t, in0=agg1, in1=a1x)
            # Horner chain for deg-6 poly (coeffs ascending in spco[0..6])
            h1 = sq_("h1")
            nc.vector.tensor_scalar(out=h1, in0=tcl, scalar1=sp(6),
                                    scalar2=sp(5), op0=ALU.mult, op1=ALU.add)
            h2 = sq_("h2")
            nc.vector._custom_dve(OPS["HORN2"], out=h2, in0=tcl, in1=h1,
                                  s0=sp(4), s1=sp(3))
            h3 = sq_("h3")
            nc.vector._custom_dve(OPS["HORN2"], out=h3, in0=tcl, in1=h2,
                                  s0=sp(2), s1=sp(1))
            s0t = sq_("s0t")
            nc.vector._custom_dve(OPS["HORN1"], out=s0t, in0=tcl, in1=h3,
                                  s0=sp(0))

            # new_sub = mixA*sub_out + mixB*sub  (per (k, ft) partition scalars)
            souts = (s0t, s1t)
            for k in range(2):
                for ft in range(2):
                    if first:
                        nc.vector.tensor_scalar(
                            out=sub_T[:, k, ft, :],
                            in0=souts[k][:, ft * P:(ft + 1) * P],
                            scalar1=mix_sb[:, k, 0, ft:ft + 1], scalar2=None,
                            op0=ALU.mult)
                    else:
                        nc.vector._custom_dve(
                            OPS["AXPBY"], out=sub_T[:, k, ft, :],
                            in0=souts[k][:, ft * P:(ft + 1) * P],
                            in1=sub_T[:, k, ft, :],
                            s0=mix_sb[:, k, 0, ft:ft + 1],
                            s1=mix_sb[:, k, 1, ft:ft + 1])

            # o = sigmoid(aggW^T [s0;s1] + aggb)
            psD = ps_scan.tile([P, 2, P], f32, name="psD", tag="psD", bufs=1)
            scat = (s0t[:, 0:P], s0t[:, P:2 * P], s1t[:, 0:P], s1t[:, P:2 * P])
            for m in range(2):
                tgt = psD[:, m, :]
                for kt in range(4):
                    nc.tensor.matmul(tgt, lhsT=aggw_sb[:, kt, m * P:(m + 1) * P],
                                     rhs=scat[kt], start=(kt == 0),
                                     stop=(kt == 3))
            o_t = stp.tile([P, 2, P], bf, name="o_t", tag="o_t")
            for m in range(2):
                nc.scalar.activation(out=o_t[:, m, :], in_=psD[:, m, :],
                                     func=ACT.Sigmoid,
                                     bias=aggb_sb[:, m:m + 1])

            # c, h updates (t1/c on gpsimd=Pool to offload DVE)
            t1 = stp.tile([P, 2, P], bf, name="t1c", tag="t1c")
            nc.gpsimd.tensor_tensor(out=t1, in0=ifg[:, 0:2, :], in1=g_t,
                                    op=ALU.mult)
            if first:
                nc.gpsimd.tensor_copy(out=c_T, in_=t1)
            else:
                t2c = stp.tile([P, 2, P], f32, name="t2c", tag="t2c")
                nc.gpsimd.tensor_tensor(out=t2c, in0=ifg[:, 2:4, :], in1=c_T,
                                        op=ALU.mult)
                nc.gpsimd.tensor_tensor(out=c_T, in0=t1, in1=t2c, op=ALU.add)
            tnc = stp.tile([P, 2, P], bf, name="tnc", tag="tnc")
            nc.scalar.activation(out=tnc, in_=c_T, func=ACT.Tanh)
            nc.vector.tensor_tensor(out=h_T, in0=o_t, in1=tnc, op=ALU.mult)

            # interleave one gather-transpose block per step (fills PE gaps)
            if gather_blocks:
                gather_block(*gather_blocks.pop(0))

        # ---- t_feat (tid-major), tid-mean centering, transpose, V ----
        scan_ps_scope.close()
        ps_v = sctx.enter_context(tc.tile_pool(name="ps_v", bufs=1, space="PSUM"))
        psT = ps_v.tile([P, D], f32, name="psT")
        nc.tensor.matmul(psT, lhsT=ones_row, rhs=projb_row, start=True,
                         stop=False)
        for kt in range(2):
            nc.tensor.matmul(psT, lhsT=h_T[:, kt, :], rhs=projw_sb[:, kt, :],
                             start=False, stop=(kt == 1))
        tf = pA.tile([P, D], f32, name="tf")
        nc.scalar.activation(out=tf, in_=psT, func=ACT.Sigmoid)

        # tid-mean via ones-matmul (contract tid partitions)
        psb = ps_v.tile([P, 2], f32, name="psb")
        for j in range(2):
            nc.tensor.matmul(psb[:, j:j + 1], lhsT=tf[:, j * P:(j + 1) * P],
                             rhs=ones_col, start=True, stop=True)
        tbar = pA.tile([P, 2], f32, name="tbar")
        nc.scalar.mul(out=tbar, in_=psb, mul=1.0 / 128.0)

        # transpose tf -> feature-major, subtract tbar
        tfT = pA.tile([P, 2, P], bf, name="tfT")
        for j in range(2):
            pt2 = ps_v.tile([P, P], f32, name="pt2", tag="pt2", bufs=2)
            nc.tensor.transpose(out=pt2, in_=tf[:, j * P:(j + 1) * P],
                                identity=ident32)
            nc.vector.tensor_scalar(out=tfT[:, j, :], in0=pt2,
                                    scalar1=tbar[:, j:j + 1], scalar2=None,
                                    op0=ALU.subtract)

        # V = tfc^T @ W0[512:768]  -> [128 tid, 1024]
        psV = ps_v.tile([P, MLP0], f32, name="psV")
        for half in range(2):
            tgt = psV[:, half * 512:(half + 1) * 512]
            for kt in range(2):
                nc.tensor.matmul(tgt, lhsT=tfT[:, kt, :],
                                 rhs=w0_sb[:, 4 + kt, half * 512:(half + 1) * 512],
                                 start=(kt == 0), stop=(kt == 1))
        nc.scalar.copy(out=V_sb[:, 0:512], in_=psV[:, 0:512])
        nc.vector.tensor_copy(out=V_sb[:, 512:1024], in_=psV[:, 512:1024])

        # leftover gather blocks
        while gather_blocks:
            gather_block(*gather_blocks.pop(0))

    # =====================================================================
    # MLP0: y0^T = W0^T z  (4 emb chunks + V@onehot), stats on the fly
    # =====================================================================
    scr = ctx.enter_context(tc.tile_pool(name="scr", bufs=2))
    ps_big = ctx.enter_context(tc.tile_pool(name="ps_big", bufs=2, space="PSUM"))

    for m in range(8):
        msl = slice(m * P, (m + 1) * P)
        bigps = ps_big.tile([P, BC], f32, name="bigps", tag="big")
        for n in range(4):
            tgt = bigps[:, n * 512:(n + 1) * 512]
            nsl = slice(n * 512, (n + 1) * 512)
            for i, kt in enumerate((0, 1, 2, 3)):
                nc.tensor.matmul(tgt, lhsT=w0_sb[:, kt, msl],
                                 rhs=zk[kt][:, nsl], start=(i == 0), stop=False)
            nc.tensor.matmul(tgt, lhsT=V_sb[:, msl], rhs=onehot_sb[:, nsl],
                             start=False, stop=True)
        nc.scalar.activation(out=y0[m], in_=bigps, func=ACT.Identity,
                             accum_out=stats0[:, m:m + 1])
        sc = scr.tile([P, BC], bf, name="sc0", tag="sc")
        nc.vector.scalar_tensor_tensor(out=sc, in0=y0[m], scalar=0.0,
                                       in1=y0[m], op0=ALU.add, op1=ALU.mult,
                                       accum_out=stats0[:, 8 + m:9 + m])

    nc.sync.dma_start(out=cc0_in, in_=stats0)
    nc.gpsimd.collective_compute(
        "AllReduce", ALU.add, replica_groups=[list(range(NCORES))],
        ins=[cc0_in.opt()], outs=[cc0_out.opt()])
    gstats0 = spool.tile([P, 16], f32, name="gstats0")
    nc.sync.dma_start(out=gstats0, in_=cc0_out)

    def bn_coefs(gstats, nm, gamma_sb, beta_sb, width):
        def t(name):
            return spool.tile([P, width], f32, name=f"{name}{nm}")
        mu = t("mu")
        nc.vector.tensor_scalar(out=mu, in0=gstats[:, 0:width], scalar1=1.0 / B,
                                scalar2=None, op0=ALU.mult)
        ey2 = t("ey2")
        nc.vector.tensor_scalar(out=ey2, in0=gstats[:, width:2 * width],
                                scalar1=1.0 / B, scalar2=None, op0=ALU.mult)
        var = t("var")
        nc.vector.scalar_tensor_tensor(out=var, in0=mu, scalar=0.0, in1=mu,
                                       op0=ALU.add, op1=ALU.mult)
        nc.vector.tensor_tensor(out=var, in0=ey2, in1=var, op=ALU.subtract)
        vpe = t("vpe")
        nc.vector.tensor_scalar(out=vpe, in0=var, scalar1=1e-5, scalar2=None,
                                op0=ALU.add)
        sd = t("sd")
        nc.scalar.activation(out=sd, in_=vpe, func=ACT.Sqrt)
        rc = t("rc")
        nc.vector.reciprocal(out=rc, in_=sd)
        tn = t("tn")
        nc.vector.tensor_tensor(out=tn, in0=vpe, in1=rc, op=ALU.mult)
        nc.vector.tensor_tensor(out=tn, in0=tn, in1=sd, op=ALU.add)
        sd2 = t("sd2")
        nc.vector.tensor_scalar(out=sd2, in0=tn, scalar1=0.5, scalar2=None,
                                op0=ALU.mult)
        rstd = t("rstd")
        nc.vector.reciprocal(out=rstd, in_=sd2)
        scale = t("scale")
        nc.vector.tensor_tensor(out=scale, in0=gamma_sb, in1=rstd, op=ALU.mult)
        shift = t("shift")
        nc.vector.tensor_tensor(out=shift, in0=mu, in1=scale, op=ALU.mult)
        nc.vector.tensor_tensor(out=shift, in0=beta_sb, in1=shift,
                                op=ALU.subtract)
        return scale, shift

    scale0, shift0 = bn_coefs(gstats0, "0", g0_sb, be0_sb, 8)
    # bn+relu column-block-wise so MLP1 can start per column block
    for n in range(4):
        nsl = slice(n * 512, (n + 1) * 512)
        for m in range(8):
            if (n + m) % 2 == 0:
                nc.scalar.activation(out=y0[m][:, nsl], in_=y0[m][:, nsl],
                                     func=ACT.Relu, bias=shift0[:, m:m + 1],
                                     scale=scale0[:, m:m + 1])
            else:
                nc.vector.tensor_scalar(out=y0[m][:, nsl], in0=y0[m][:, nsl],
                                        scalar1=scale0[:, m:m + 1],
                                        scalar2=shift0[:, m:m + 1],
                                        op0=ALU.mult, op1=ALU.add)
                nc.vector.tensor_scalar(out=y0[m][:, nsl], in0=y0[m][:, nsl],
                                        scalar1=0.0, scalar2=None, op0=ALU.max)

    # =====================================================================
    # MLP1, stats, AR1, bn+relu, fc, output
    # =====================================================================
    y1 = [zpool.tile([P, BC], bf, name=f"y1_{m}") for m in range(2)]
    for m in range(2):
        msl = slice(m * P, (m + 1) * P)
        bigps = ps_big.tile([P, BC], f32, name="bigps1", tag="big")
        for n in range(4):
            tgt = bigps[:, n * 512:(n + 1) * 512]
            nsl = slice(n * 512, (n + 1) * 512)
            for kt in range(8):
                nc.tensor.matmul(tgt, lhsT=w1_sb[:, kt, msl],
                                 rhs=y0[kt][:, nsl], start=(kt == 0),
                                 stop=(kt == 7))
        nc.scalar.activation(out=y1[m], in_=bigps, func=ACT.Identity,
                             accum_out=stats1[:, m:m + 1])
        sc = scr.tile([P, BC], bf, name="sc1", tag="sc")
        nc.vector.scalar_tensor_tensor(out=sc, in0=y1[m], scalar=0.0,
                                       in1=y1[m], op0=ALU.add, op1=ALU.mult,
                                       accum_out=stats1[:, 2 + m:3 + m])

    nc.sync.dma_start(out=cc1_in, in_=stats1)
    nc.gpsimd.collective_compute(
        "AllReduce", ALU.add, replica_groups=[list(range(NCORES))],
        ins=[cc1_in.opt()], outs=[cc1_out.opt()])
    gstats1 = spool.tile([P, 4], f32, name="gstats1")
    nc.sync.dma_start(out=gstats1, in_=cc1_out)

    scale1, shift1 = bn_coefs(gstats1, "1", g1_sb, be1_sb, 2)
    nc.scalar.activation(out=y1[0], in_=y1[0], func=ACT.Relu,
                         bias=shift1[:, 0:1], scale=scale1[:, 0:1])
    nc.vector.tensor_scalar(out=y1[1], in0=y1[1], scalar1=scale1[:, 1:2],
                            scalar2=shift1[:, 1:2], op0=ALU.mult, op1=ALU.add)
    nc.vector.tensor_scalar(out=y1[1], in0=y1[1], scalar1=0.0, scalar2=None,
                            op0=ALU.max)

    out_sb = spool.tile([1, BC], f32, name="out_sb")
    for n in range(4):
        nsl = slice(n * 512, (n + 1) * 512)
        psf = ps_big.tile([1, 512], f32, name="psf", tag="big")
        for kt in range(2):
            nc.tensor.matmul(psf, lhsT=fcw_sb[:, kt, :], rhs=y1[kt][:, nsl],
                             start=(kt == 0), stop=(kt == 1))
        nc.scalar.activation(out=out_sb[:, nsl], in_=psf, func=ACT.Identity,
                             bias=fcb_sb[0:1, 0:1])
    nc.sync.dma_start(out=out_d, in_=out_sb)


# ---------------- module build + run -----------------------------------------
_CACHE = {}


def build_module():
    from contextlib import ExitStack
    import concourse.bacc as bacc
    import concourse.tile as tile
    _register_dve_ops()
    nc = bacc.Bacc("TRN2", target_bir_lowering=False, debug=False,
                   num_devices=NCORES)
    with tile.TileContext(nc) as tc:
        with ExitStack() as ctx:
            _emit(nc, tc, ctx)
    nc.compile()
    return nc


def _get_module():
    if "nc" not in _CACHE:
        _CACHE["nc"] = build_module()
    return _CACHE["nc"]


def host_prep(inputs):
    """Build per-core input maps from the full input dict."""
    gi = {k: np.asarray(v) for k, v in inputs.items()}
    P = 128
    x = gi["x"].astype(np.int64)
    t_id = x[:, 0]
    u_id = np.clip(x[:, 1], 0, EMB_ROWS - 1).astype(np.int32)
    i_id = np.clip(x[:, 2] + NUM_USERS, 0, EMB_ROWS - 1).astype(np.int32)

    emb = np.asarray(gi["embedding"], np.float32)
    T = emb[TROW0:TROW0 + NTROWS]                      # (135, 256)
    # host-folded x-parts of the cell (fp32), -> [p, chunk, col] bf16
    G = (T @ np.asarray(gi["Wg"], np.float32)
         + np.asarray(gi["bg"], np.float32)[None, :])  # (135, 768)
    A = np.einsum("cf,kfe->cke", T,
                  np.asarray(gi["sub_Wx"], np.float32)) # (135, 2, 256)
    gpre = np.ascontiguousarray(
        G.T.reshape(6, P, NTROWS).transpose(1, 0, 2)).astype(BF16)
    # A[c, k, e] -> chunks (k, ftchunk): [p, k*2+ft, c]
    apre = np.ascontiguousarray(
        A.transpose(1, 2, 0).reshape(2, 2, P, NTROWS)
        .transpose(2, 0, 1, 3).reshape(P, 4, NTROWS)).astype(BF16)

    def kchunks(w, nk, width):                          # (nk*128, width) -> [p, nk, width]
        return np.ascontiguousarray(
            np.asarray(w, np.float32).reshape(nk, P, width)
            .transpose(1, 0, 2)).astype(BF16)

    ug16 = kchunks(gi["Ug"], 2, 3 * D)
    wh16 = np.ascontiguousarray(
        np.asarray(gi["sub_Wh"], np.float32).reshape(2, 2, P, D)
        .transpose(2, 0, 1, 3)).astype(BF16)            # [p, k, kt, e]
    aggw16 = kchunks(gi["aggW"], 4, D)
    projw16 = kchunks(gi["projW"], 2, D)
    w0_16 = kchunks(gi["W0"], 6, MLP0)
    w1_16 = kchunks(gi["W1"], 8, MLP1)
    fcw16 = kchunks(gi["fcW"], 2, 1)

    def pcol(v, nt):                                    # (nt*128,) -> [p, nt]
        return np.ascontiguousarray(
            np.asarray(v, np.float32).reshape(nt, P).T)

    mix = np.asarray(gi["sub_mix"], np.float32)         # (2, 2, 256)
    mixsb = np.ascontiguousarray(
        mix.reshape(2, 2, 2, P).transpose(3, 0, 1, 2))  # [p, k, j, ft]

    pc = _fit_poly(gi["spline_coef"], CFG["poly_deg"])  # ascending, deg+1
    spco = np.zeros((1, 8), np.float32)
    spco[0, :len(pc)] = pc

    shared = dict(
        emb16=np.ascontiguousarray(emb.astype(BF16)),
        gpre=gpre, apre=apre, ug=ug16, wh=wh16, aggw=aggw16,
        projw=projw16, w0=w0_16, w1=w1_16, fcw=fcw16,
        aggb=pcol(gi["aggb"], 2),
        projbrow=np.ascontiguousarray(
            np.asarray(gi["projb"], np.float32)[None, :]).astype(BF16),
        mixsb=mixsb, spco=spco,
        gamma0=pcol(gi["gamma0"], 8), beta0=pcol(gi["beta0"], 8),
        gamma1=pcol(gi["gamma1"], 2), beta1=pcol(gi["beta1"], 2),
        fcb=np.asarray(gi["fcb"], np.float32).reshape(1, 1),
    )

    in_maps = []
    for c in range(NCORES):
        rows = slice(c * BC, (c + 1) * BC)
        tid_c = t_id[rows]
        onehot = np.zeros((P, BC), BF16)
        onehot[tid_c, np.arange(BC)] = 1.0
        m = dict(shared)
        m["onehot"] = onehot
        m["uidx"] = np.ascontiguousarray(u_id[rows].reshape(NB, P).T)
        m["iidx"] = np.ascontiguousarray(i_id[rows].reshape(NB, P).T)
        in_maps.append(m)
    return in_maps


def kernel(**inputs):
    from concourse import bass_utils
    nc = _get_module()
    in_maps = host_prep(inputs)
    res = bass_utils.run_bass_kernel_spmd(
        nc, in_maps, core_ids=list(range(NCORES)))
    _CACHE["last_results"] = res
    out = np.concatenate([res.results[c]["out"] for c in range(NCORES)])
    return out.astype(np.float32)


# revision 30
# speedup vs baseline: 1.9519x; 1.0288x over previous
"""Trainium2 Bass kernel for nn_NeuralTensorFactorization (8-core SPMD), v2.

Design (validated numerically in proto.py, rel err ~5.5e-3 vs fp64, tol 2e-2):
- Scan over the 128 distinct tids (batch-independent), feature-major, bf16
  matmuls (4x over f32), fp32 psum/c-state. x-dependent parts of the cell
  (x@Wg+bg, x@Wx) are host-folded into G (128,6,135) / A (128,4,135) tables
  (linear weight folding over the 135 embedding time rows); the device scan
  keeps the recurrence: Ug*h, Wh*sub, spline, mix, aggW, LSTM cell.
- Spline -> degree-6 poly (host minimax fit), evaluated with custom fused DVE
  ops (CLAMPSUM chain) ~5 DVE ops vs ~21 stock.
- Batch 16384 sharded 2048/core; u/i embeddings gathered from a bf16 copy of
  the table, PE-transposed to feature-major bf16 zk tiles.
- MLP0 via 5 matmuls/chunk: 4 emb chunks + V@onehot where V = tfc^T @ W0[512:]
  (associativity; replaces separate one-hot gather of t_feat).
- BN: per-feature (=partition) sum/sumsq accumulated on psum->sbuf copy (Act)
  and square (DVE), two tiny AllReduces; t_feat centered by tid-mean to avoid
  fp32 variance cancellation. bf16 y0/y1 storage.
"""

import numpy as np
import ml_dtypes

# ---------------- model constants (hardcoded; must match the problem) --------
NUM_TIMES, NUM_USERS, NUM_ITEMS = 128, 5000, 50000
STEP, D, B = 8, 256, 16384
MLP0, MLP1 = 1024, 256
NCORES = 8
BC = B // NCORES              # 2048 rows per core
NB = BC // 128                # 16 batch chunks of 128
TROW0 = 55000                 # first embedding row used by the scan windows
NTROWS = NUM_TIMES + STEP - 1 # 135 rows
EMB_ROWS = NUM_TIMES + NUM_USERS + NUM_ITEMS + STEP  # 55136

CFG = dict(
    probes=False,
    poly_deg=6,
    batched_gather=False,   # one indirect DMA per source vs per-128-row chunk
    seed_mode="mm3d",       # "mm3d" | "mm2d" | "dve": how G/A enter psum
)

BF16 = ml_dtypes.bfloat16

# ---------------- host-side spline poly fit (fp64) ---------------------------
SPLINE_ORDER, GRID_SIZE = 3, 5
_H = 2.0 / GRID_SIZE
_KNOTS = np.arange(-SPLINE_ORDER, GRID_SIZE + SPLINE_ORDER + 1,
                   dtype=np.float64) * _H - 1.0


def _bspline_basis64(x):
    knots = _KNOTS
    xe = np.asarray(x, np.float64)[..., None]
    b = ((xe >= knots[:-1]) & (xe < knots[1:])).astype(np.float64)
    for d in range(1, SPLINE_ORDER + 1):
        left = (xe - knots[:-(d + 1)]) / (knots[d:-1] - knots[:-(d + 1)]) * b[..., :-1]
        right = (knots[d + 1:] - xe) / (knots[d + 1:] - knots[1:-d]) * b[..., 1:]
        b = left + right
    return b


def _fit_poly(coef, deg):
    """Weighted LS fit of the clamped spline on [-1,1]; returns c[k] = coeff
    of t^k, ascending, float32, length deg+1."""
    xs = np.linspace(-1.0, 1.0, 4001)
    ys = _bspline_basis64(xs) @ np.asarray(coef, np.float64)
    w = np.ones_like(xs)
    for b in (-1.0, -0.6, -0.2, 0.2, 0.6, 1.0):
        w += 2.0 * np.exp(-((xs - b) / 0.05) ** 2)
    V = np.vander(xs, deg + 1)            # highest power first
    c = np.linalg.lstsq(V * w[:, None], ys * w, rcond=None)[0]
    return c[::-1].astype(np.float32)     # ascending


# ---------------- custom DVE ops ---------------------------------------------
_DVE_OPS = {}


def _register_dve_ops():
    """Idempotently register the fused DVE ops this kernel uses."""
    if _DVE_OPS:
        return _DVE_OPS
    from concourse import dve_ops as DOPS
    from concourse.dve_spec import (Spec, Src0, Src1, C0, C1, One, Zero,
                                    maxx, minn, sq, lower)
    from concourse.dve_uop import DveOpSpec

    def reg(name, spec):
        if name in DOPS._SUB_OPCODE_FOR_NAME:
            return next(o for o in DOPS.OPS if o.name == name)
        row = DOPS._CUSTOM_DVE_ROW_BASE + len(DOPS.OPS)
        shas = {}
        for v in ("v3", "v4"):
            uops = lower(spec, ver=v)
            shas[v] = DveOpSpec(name=name, opcode=row, uops=uops,
                                rd1_en=DOPS.has_src1(spec)).sha(v)
        op = DOPS.DveOp(name, spec, subdim=False, uops_sha=shas)
        DOPS._SUB_OPCODE_FOR_NAME[name] = row
        DOPS.OPS.append(op)
        DOPS.CUSTOM_DVE_SPECS[name] = spec
        return op

    def _clip(in0, imm2):
        return np.minimum(np.maximum(in0.astype(np.float32), imm2), 1.0)

    # t = clip(in0 + in1, -1, 1)
    _DVE_OPS["CLAMPSUM"] = reg("NTF_CLAMPSUM", Spec(
        body=minn(maxx(Src0 + Src1, Zero - One), One),
        reference=lambda in0, in1, s0, s1, imm2:
            _clip(in0 + in1, -1.0)))
    # out = (in0 + in1)^2
    _DVE_OPS["SUMSQ"] = reg("NTF_SUMSQ", Spec(
        body=sq(Src0 + Src1),
        reference=lambda in0, in1, s0, s1, imm2:
            ((in0.astype(np.float32) + in1) ** 2)))
    # Horner step: out = (in1*in0 + s0)*in0 + s1   (in0 = t, in1 = chain)
    _DVE_OPS["HORN2"] = reg("NTF_HORN2", Spec(
        body=(Src1 * Src0 + C0) * Src0 + C1,
        reference=lambda in0, in1, s0, s1, imm2:
            ((in1 * in0.astype(np.float32) + s0) * in0 + s1)))
    # out = in1*in0 + s0
    _DVE_OPS["HORN1"] = reg("NTF_HORN1", Spec(
        body=Src1 * Src0 + C0,
        reference=lambda in0, in1, s0, s1, imm2:
            (in1 * in0.astype(np.float32) + s0)))
    # out = in0*s0 + in1*s1
    _DVE_OPS["AXPBY"] = reg("NTF_AXPBY", Spec(
        body=Src0 * C0 + Src1 * C1,
        reference=lambda in0, in1, s0, s1, imm2:
            (in0.astype(np.float32) * s0 + in1 * s1)))
    return _DVE_OPS


# ---------------- device program ---------------------------------------------

def _emit(nc, tc, ctx):
    import concourse.bass as bass
    import concourse.mybir as mybir
    from concourse.masks import make_identity
    from contextlib import ExitStack
    OPS = _register_dve_ops()
    dt = mybir.dt
    f32 = dt.float32
    bf = dt.bfloat16
    ACT = mybir.ActivationFunctionType
    ALU = mybir.AluOpType
    P = 128

    # ---- DRAM I/O ----
    def din(name, shape, dtype=f32):
        return nc.dram_tensor(name, shape, dtype, kind="ExternalInput").ap()

    emb16 = din("emb16", (EMB_ROWS, D), bf)
    onehot_d = din("onehot", (P, BC), bf)
    uidx_d = din("uidx", (P, NB), dt.int32)
    iidx_d = din("iidx", (P, NB), dt.int32)
    g_d = din("gpre", (P, 6, NTROWS), bf)        # host-folded Wg^T.x + bg
    a_d = din("apre", (P, 4, NTROWS), bf)        # host-folded Wx^T.x
    ug_d = din("ug", (P, 2, 3 * D), bf)
    wh_d = din("wh", (P, 2, 2, D), bf)
    aggw_d = din("aggw", (P, 4, D), bf)
    projw_d = din("projw", (P, 2, D), bf)
    w0_d = din("w0", (P, 6, MLP0), bf)
    w1_d = din("w1", (P, 8, MLP1), bf)
    fcw_d = din("fcw", (P, 2, 1), bf)
    aggb_d = din("aggb", (P, 2))
    projb_d = din("projbrow", (1, D), bf)
    mix_d = din("mixsb", (P, 2, 2, 2))
    spco_d = din("spco", (1, 8))
    g0_d = din("gamma0", (P, 8)); be0_d = din("beta0", (P, 8))
    g1_d = din("gamma1", (P, 2)); be1_d = din("beta1", (P, 2))
    fcb_d = din("fcb", (1, 1))

    out_d = nc.dram_tensor("out", (BC,), f32, kind="ExternalOutput").ap()

    ccw_in = nc.dram_tensor("ccw_in", (P, 1), f32).ap()
    ccw_out = nc.dram_tensor("ccw_out", (P, 1), f32, addr_space="Shared").ap()
    cc0_in = nc.dram_tensor("cc0_in", (P, 16), f32).ap()
    cc0_out = nc.dram_tensor("cc0_out", (P, 16), f32, addr_space="Shared").ap()
    cc1_in = nc.dram_tensor("cc1_in", (P, 4), f32).ap()
    cc1_out = nc.dram_tensor("cc1_out", (P, 4), f32, addr_space="Shared").ap()

    # ---- whole-kernel pools ----
    consts = ctx.enter_context(tc.tile_pool(name="consts", bufs=1))
    zpool = ctx.enter_context(tc.tile_pool(name="zpool", bufs=1))

    def load(pool, name, dram_ap, shape, dtype=f32):
        t = pool.tile(shape, dtype, name=name)
        nc.sync.dma_start(out=t, in_=dram_ap)
        return t

    # scan-critical weights first (small; scan can start ~immediately)
    g_sb = load(consts, "g_sb", g_d, [P, 6, NTROWS], bf)
    a_sb = load(consts, "a_sb", a_d, [P, 4, NTROWS], bf)
    ug_sb = load(consts, "ug_sb", ug_d, [P, 2, 3 * D], bf)
    wh_sb = load(consts, "wh_sb", wh_d, [P, 2, 2, D], bf)
    aggw_sb = load(consts, "aggw_sb", aggw_d, [P, 4, D], bf)
    aggb_sb = load(consts, "aggb_sb", aggb_d, [P, 2])
    mix_sb = load(consts, "mix_sb", mix_d, [P, 2, 2, 2])
    spco = load(consts, "spco", spco_d.to_broadcast([P, 8]), [P, 8])
    projw_sb = load(consts, "projw_sb", projw_d, [P, 2, D], bf)
    projb_row = load(consts, "projb_row", projb_d, [1, D], bf)
    uidx_sb = load(consts, "uidx_sb", uidx_d, [P, NB], dt.int32)
    iidx_sb = load(consts, "iidx_sb", iidx_d, [P, NB], dt.int32)
    # big weights stream while the scan runs
    w0_sb = load(consts, "w0_sb", w0_d, [P, 6, MLP0], bf)
    onehot_sb = load(consts, "onehot_sb", onehot_d, [P, BC], bf)
    w1_sb = load(consts, "w1_sb", w1_d, [P, 8, MLP1], bf)
    fcw_sb = load(consts, "fcw_sb", fcw_d, [P, 2, 1], bf)
    g0_sb = load(consts, "g0_sb", g0_d, [P, 8])
    be0_sb = load(consts, "be0_sb", be0_d, [P, 8])
    g1_sb = load(consts, "g1_sb", g1_d, [P, 2])
    be1_sb = load(consts, "be1_sb", be1_d, [P, 2])
    fcb_sb = load(consts, "fcb_sb", fcb_d, [1, 1])

    ident16 = consts.tile([P, P], bf, name="ident16")
    make_identity(nc, ident16)
    ident32 = consts.tile([P, P], f32, name="ident32")
    make_identity(nc, ident32)
    ones_col = consts.tile([P, 1], f32, name="ones_col")
    nc.vector.memset(ones_col, 1.0)
    ones_row = consts.tile([1, P], bf, name="ones_row")
    nc.vector.memset(ones_row, 1.0)

    # collective warmup (absorbs first-collective cold-start)
    warm_sb = consts.tile([P, 1], f32, name="warm_sb")
    nc.vector.memset(warm_sb, 1.0)
    nc.sync.dma_start(out=ccw_in, in_=warm_sb)
    nc.gpsimd.collective_compute(
        "AllReduce", ALU.add, replica_groups=[list(range(NCORES))],
        ins=[ccw_in.opt()], outs=[ccw_out.opt()])

    def sp(j):  # poly coefficient j as per-partition scalar AP
        return spco[:, j:j + 1]

    zk = [zpool.tile([P, BC], bf, name=f"zk{i}") for i in range(4)]
    y0 = [zpool.tile([P, BC], bf, name=f"y0_{m}") for m in range(8)]
    V_sb = zpool.tile([P, MLP0], bf, name="V_sb")

    spool = ctx.enter_context(tc.tile_pool(name="spool", bufs=1))
    stats0 = spool.tile([P, 16], f32, name="stats0")
    stats1 = spool.tile([P, 4], f32, name="stats1")

    # =====================================================================
    # Scan + gathers (scoped pools released before the MLP)
    # =====================================================================
    with ExitStack() as sctx:
        pA = sctx.enter_context(tc.tile_pool(name="pA", bufs=1))
        stp = sctx.enter_context(tc.tile_pool(name="stp", bufs=2))
        ps_tr = sctx.enter_context(tc.tile_pool(name="ps_tr", bufs=2, space="PSUM"))
        scan_ps_scope = ExitStack()
        ps_scan = scan_ps_scope.enter_context(
            tc.tile_pool(name="ps_scan", bufs=1, space="PSUM"))

        # persistent scan state, feature-major
        h_T = pA.tile([P, 2, P], bf, name="h_T")
        c_T = pA.tile([P, 2, P], f32, name="c_T")
        sub_T = pA.tile([P, 2, 2, P], bf, name="sub_T")

        # ---- gather u/i embedding rows -> [P, NB, D] tiles ----
        gtiles = []
        for src, idx_sb in ((0, uidx_sb), (1, iidx_sb)):
            t = pA.tile([P, NB, D], bf, name=f"gt{src}")
            if CFG["batched_gather"]:
                nc.gpsimd.indirect_dma_start(
                    out=t, out_offset=None, in_=emb16,
                    in_offset=bass.IndirectOffsetOnAxis(ap=idx_sb, axis=0))
            else:
                for c in range(NB):
                    nc.gpsimd.indirect_dma_start(
                        out=t[:, c, :], out_offset=None, in_=emb16,
                        in_offset=bass.IndirectOffsetOnAxis(
                            ap=idx_sb[:, c:c + 1], axis=0))
            gtiles.append(t)

        def gather_block(src, cb):
            # 4 chunks x 2 ft -> two [P,512] psum banks -> 2 copies into zk
            for ft in range(2):
                pt = ps_tr.tile([P, 512], bf, name="pt", tag="pt")
                for j in range(4):
                    c = cb * 4 + j
                    nc.tensor.transpose(out=pt[:, j * P:(j + 1) * P],
                                        in_=gtiles[src][:, c, ft * P:(ft + 1) * P],
                                        identity=ident16)
                dst = zk[2 * src + ft][:, cb * 512:(cb + 1) * 512]
                if (src + ft + cb) % 2 == 0:
                    nc.scalar.copy(out=dst, in_=pt)
                else:
                    nc.vector.tensor_copy(out=dst, in_=pt)

        gather_blocks = [(s, cb) for s in range(2) for cb in range(4)]

        # ---- the scan ----
        GS = lambda m0, m1, s: g_sb[:, m0:m1, s:s + P]
        AS = lambda k0, k1, s: a_sb[:, k0:k1, s:s + P]

        for s in range(STEP):
            first = s == 0
            # gates psum seeded with host-folded x@Wg+bg via identity matmul
            psA = ps_scan.tile([P, 4, P], f32, name="psA", tag="psA", bufs=2)
            psB = ps_scan.tile([P, 2, P], f32, name="psB", tag="psB", bufs=1)
            if CFG["seed_mode"] == "mm3d":
                nc.tensor.matmul(psA, lhsT=ident16, rhs=GS(0, 4, s), start=True,
                                 stop=first, skip_group_check=True)
                nc.tensor.matmul(psB, lhsT=ident16, rhs=GS(4, 6, s), start=True,
                                 stop=first, skip_group_check=True)
            else:
                for m in range(6):
                    tgt = psA[:, m, :] if m < 4 else psB[:, m - 4, :]
                    nc.tensor.matmul(tgt, lhsT=ident16,
                                     rhs=g_sb[:, m, s:s + P], start=True,
                                     stop=first, skip_group_check=True)
            if not first:
                for m in range(6):
                    tgt = psA[:, m, :] if m < 4 else psB[:, m - 4, :]
                    for kt in range(2):
                        nc.tensor.matmul(tgt, lhsT=ug_sb[:, kt, m * P:(m + 1) * P],
                                         rhs=h_T[:, kt, :], start=False,
                                         stop=(kt == 1), skip_group_check=True)
            ifg = stp.tile([P, 4, P], bf, name="ifg", tag="ifg")
            nc.scalar.activation(out=ifg, in_=psA, func=ACT.Sigmoid)
            g_t = stp.tile([P, 2, P], bf, name="g_t", tag="g_t")
            nc.scalar.activation(out=g_t, in_=psB, func=ACT.Tanh)

            psC = ps_scan.tile([P, 2, 2, P], f32, name="psC", tag="psC",
                               bufs=2)
            if CFG["seed_mode"] == "mm3d":
                nc.tensor.matmul(psC, lhsT=ident16, rhs=AS(0, 4, s), start=True,
                                 stop=first, skip_group_check=True)
            else:
                for k in range(2):
                    for mf in range(2):
                        nc.tensor.matmul(psC[:, k, mf, :], lhsT=ident16,
                                         rhs=a_sb[:, 2 * k + mf, s:s + P],
                                         start=True, stop=first,
                                         skip_group_check=True)
            if not first:
                for k in range(2):
                    for mf in range(2):
                        tgt = psC[:, k, mf, :]
                        for kt in range(2):
                            nc.tensor.matmul(
                                tgt, lhsT=wh_sb[:, k, kt, mf * P:(mf + 1) * P],
                                rhs=sub_T[:, k, kt, :], start=False,
                                stop=(kt == 1), skip_group_check=True)

            sq_ = lambda nm: stp.tile([P, 256], bf, name=nm, tag=nm)
            # spline input t = clip(agg0, -1, 1); s1 = agg1^2
            tcl = sq_("tcl")
            nc.vector.tensor_scalar(out=tcl, in0=psC[:, 0], scalar1=1.0,
                                    scalar2=-1.0, op0=ALU.min, op1=ALU.max)
            s1t = sq_("s1t")
            nc.scalar.activation(out=s1t, in_=psC[:, 1], func=ACT.Square)
            # Horner chain for deg-6 poly (coeffs ascending in spco[0..6])
            h1 = sq_("h1")
            nc.vector.tensor_scalar(out=h1, in0=tcl, scalar1=sp(6),
                                    scalar2=sp(5), op0=ALU.mult, op1=ALU.add)
            h2 = sq_("h2")
            nc.vector._custom_dve(OPS["HORN2"], out=h2, in0=tcl, in1=h1,
                                  s0=sp(4), s1=sp(3))
            h3 = sq_("h3")
            nc.vector._custom_dve(OPS["HORN2"], out=h3, in0=tcl, in1=h2,
                                  s0=sp(2), s1=sp(1))
            s0t = sq_("s0t")
            nc.vector._custom_dve(OPS["HORN1"], out=s0t, in0=tcl, in1=h3,
                                  s0=sp(0))

            # new_sub = mixA*sub_out + mixB*sub  (per (k, ft) partition scalars)
            souts = (s0t, s1t)
            for k in range(2):
                for ft in range(2):
                    if first:
                        nc.vector.tensor_scalar(
                            out=sub_T[:, k, ft, :],
                            in0=souts[k][:, ft * P:(ft + 1) * P],
                            scalar1=mix_sb[:, k, 0, ft:ft + 1], scalar2=None,
                            op0=ALU.mult)
                    else:
                        nc.vector._custom_dve(
                            OPS["AXPBY"], out=sub_T[:, k, ft, :],
                            in0=souts[k][:, ft * P:(ft + 1) * P],
                            in1=sub_T[:, k, ft, :],
                            s0=mix_sb[:, k, 0, ft:ft + 1],
                            s1=mix_sb[:, k, 1, ft:ft + 1])

            # o = sigmoid(aggW^T [s0;s1] + aggb)
            psD = ps_scan.tile([P, 2, P], f32, name="psD", tag="psD", bufs=1)
            scat = (s0t[:, 0:P], s0t[:, P:2 * P], s1t[:, 0:P], s1t[:, P:2 * P])
            for m in range(2):
                tgt = psD[:, m, :]
                for kt in range(4):
                    nc.tensor.matmul(tgt, lhsT=aggw_sb[:, kt, m * P:(m + 1) * P],
                                     rhs=scat[kt], start=(kt == 0),
                                     stop=(kt == 3))
            o_t = stp.tile([P, 2, P], bf, name="o_t", tag="o_t")
            for m in range(2):
                nc.scalar.activation(out=o_t[:, m, :], in_=psD[:, m, :],
                                     func=ACT.Sigmoid,
                                     bias=aggb_sb[:, m:m + 1])

            # c, h updates (t1/c on gpsimd=Pool to offload DVE)
            t1 = stp.tile([P, 2, P], bf, name="t1c", tag="t1c")
            nc.gpsimd.tensor_tensor(out=t1, in0=ifg[:, 0:2, :], in1=g_t,
                                    op=ALU.mult)
            if first:
                nc.gpsimd.tensor_copy(out=c_T, in_=t1)
            else:
                t2c = stp.tile([P, 2, P], f32, name="t2c", tag="t2c")
                nc.gpsimd.tensor_tensor(out=t2c, in0=ifg[:, 2:4, :], in1=c_T,
                                        op=ALU.mult)
                nc.gpsimd.tensor_tensor(out=c_T, in0=t1, in1=t2c, op=ALU.add)
            tnc = stp.tile([P, 2, P], bf, name="tnc", tag="tnc")
            nc.scalar.activation(out=tnc, in_=c_T, func=ACT.Tanh)
            nc.gpsimd.tensor_tensor(out=h_T, in0=o_t, in1=tnc, op=ALU.mult)

            # interleave one gather-transpose block per step (fills PE gaps)
            if gather_blocks:
                gather_block(*gather_blocks.pop(0))

        # ---- t_feat (tid-major), tid-mean centering, transpose, V ----
        scan_ps_scope.close()
        ps_v = sctx.enter_context(tc.tile_pool(name="ps_v", bufs=1, space="PSUM"))
        psT = ps_v.tile([P, D], f32, name="psT")
        nc.tensor.matmul(psT, lhsT=ones_row, rhs=projb_row, start=True,
                         stop=False)
        for kt in range(2):
            nc.tensor.matmul(psT, lhsT=h_T[:, kt, :], rhs=projw_sb[:, kt, :],
                             start=False, stop=(kt == 1))
        tf = pA.tile([P, D], f32, name="tf")
        nc.scalar.activation(out=tf, in_=psT, func=ACT.Sigmoid)

        # tid-mean via ones-matmul (contract tid partitions)
        psb = ps_v.tile([P, 2], f32, name="psb")
        for j in range(2):
            nc.tensor.matmul(psb[:, j:j + 1], lhsT=tf[:, j * P:(j + 1) * P],
                             rhs=ones_col, start=True, stop=True)
        tbar = pA.tile([P, 2], f32, name="tbar")
        nc.scalar.mul(out=tbar, in_=psb, mul=1.0 / 128.0)

        # transpose tf -> feature-major, subtract tbar
        tfT = pA.tile([P, 2, P], bf, name="tfT")
        for j in range(2):
            pt2 = ps_v.tile([P, P], f32, name="pt2", tag="pt2", bufs=2)
            nc.tensor.transpose(out=pt2, in_=tf[:, j * P:(j + 1) * P],
                                identity=ident32)
            nc.vector.tensor_scalar(out=tfT[:, j, :], in0=pt2,
                                    scalar1=tbar[:, j:j + 1], scalar2=None,
                                    op0=ALU.subtract)

        # V = tfc^T @ W0[512:768]  -> [128 tid, 1024]
        psV = ps_v.tile([P, MLP0], f32, name="psV")
        for half in range(2):
            tgt = psV[:, half * 512:(half + 1) * 512]
            for kt in range(2):
                nc.tensor.matmul(tgt, lhsT=tfT[:, kt, :],
                                 rhs=w0_sb[:, 4 + kt, half * 512:(half + 1) * 512],
                                 start=(kt == 0), stop=(kt == 1))
        nc.scalar.copy(out=V_sb[:, 0:512], in_=psV[:, 0:512])
        nc.vector.tensor_copy(out=V_sb[:, 512:1024], in_=psV[:, 512:1024])

        # leftover gather blocks
        while gather_blocks:
            gather_block(*gather_blocks.pop(0))

    # =====================================================================
    # MLP0: y0^T = W0^T z  (4 emb chunks + V@onehot), stats on the fly
    # =====================================================================
    scr = ctx.enter_context(tc.tile_pool(name="scr", bufs=2))
    ps_big = ctx.enter_context(tc.tile_pool(name="ps_big", bufs=2, space="PSUM"))

    for m in range(8):
        msl = slice(m * P, (m + 1) * P)
        bigps = ps_big.tile([P, BC], f32, name="bigps", tag="big")
        for n in range(4):
            tgt = bigps[:, n * 512:(n + 1) * 512]
            nsl = slice(n * 512, (n + 1) * 512)
            for i, kt in enumerate((0, 1, 2, 3)):
                nc.tensor.matmul(tgt, lhsT=w0_sb[:, kt, msl],
                                 rhs=zk[kt][:, nsl], start=(i == 0), stop=False)
            nc.tensor.matmul(tgt, lhsT=V_sb[:, msl], rhs=onehot_sb[:, nsl],
                             start=False, stop=True)
        nc.scalar.activation(out=y0[m], in_=bigps, func=ACT.Identity,
                             accum_out=stats0[:, m:m + 1])
        sc = scr.tile([P, BC], bf, name="sc0", tag="sc")
        nc.vector.scalar_tensor_tensor(out=sc, in0=y0[m], scalar=0.0,
                                       in1=y0[m], op0=ALU.add, op1=ALU.mult,
                                       accum_out=stats0[:, 8 + m:9 + m])

    nc.sync.dma_start(out=cc0_in, in_=stats0)
    nc.gpsimd.collective_compute(
        "AllReduce", ALU.add, replica_groups=[list(range(NCORES))],
        ins=[cc0_in.opt()], outs=[cc0_out.opt()])
    gstats0 = spool.tile([P, 16], f32, name="gstats0")
    nc.sync.dma_start(out=gstats0, in_=cc0_out)

    def bn_coefs(gstats, nm, gamma_sb, beta_sb, width):
        def t(name):
            return spool.tile([P, width], f32, name=f"{name}{nm}")
        mu = t("mu")
        nc.vector.tensor_scalar(out=mu, in0=gstats[:, 0:width], scalar1=1.0 / B,
                                scalar2=None, op0=ALU.mult)
        ey2 = t("ey2")
        nc.vector.tensor_scalar(out=ey2, in0=gstats[:, width:2 * width],
                                scalar1=1.0 / B, scalar2=None, op0=ALU.mult)
        var = t("var")
        nc.vector.scalar_tensor_tensor(out=var, in0=mu, scalar=0.0, in1=mu,
                                       op0=ALU.add, op1=ALU.mult)
        nc.vector.tensor_tensor(out=var, in0=ey2, in1=var, op=ALU.subtract)
        vpe = t("vpe")
        nc.vector.tensor_scalar(out=vpe, in0=var, scalar1=1e-5, scalar2=None,
                                op0=ALU.add)
        sd = t("sd")
        nc.scalar.activation(out=sd, in_=vpe, func=ACT.Sqrt)
        rstd = t("rstd")
        nc.vector.reciprocal(out=rstd, in_=sd)
        scale = t("scale")
        nc.vector.tensor_tensor(out=scale, in0=gamma_sb, in1=rstd, op=ALU.mult)
        shift = t("shift")
        nc.vector.tensor_tensor(out=shift, in0=mu, in1=scale, op=ALU.mult)
        nc.vector.tensor_tensor(out=shift, in0=beta_sb, in1=shift,
                                op=ALU.subtract)
        return scale, shift

    scale0, shift0 = bn_coefs(gstats0, "0", g0_sb, be0_sb, 8)
    # bn+relu column-block-wise so MLP1 can start per column block
    for n in range(4):
        nsl = slice(n * 512, (n + 1) * 512)
        for m in range(8):
            if (n + m) % 2 == 0:
                nc.scalar.activation(out=y0[m][:, nsl], in_=y0[m][:, nsl],
                                     func=ACT.Relu, bias=shift0[:, m:m + 1],
                                     scale=scale0[:, m:m + 1])
            else:
                nc.vector.tensor_scalar(out=y0[m][:, nsl], in0=y0[m][:, nsl],
                                        scalar1=scale0[:, m:m + 1],
                                        scalar2=shift0[:, m:m + 1],
                                        op0=ALU.mult, op1=ALU.add)
                nc.vector.tensor_scalar(out=y0[m][:, nsl], in0=y0[m][:, nsl],
                                        scalar1=0.0, scalar2=None, op0=ALU.max)

    # =====================================================================
    # MLP1, stats, AR1, bn+relu, fc, output
    # =====================================================================
    y1 = [zpool.tile([P, BC], bf, name=f"y1_{m}") for m in range(2)]
    for m in range(2):
        msl = slice(m * P, (m + 1) * P)
        bigps = ps_big.tile([P, BC], f32, name="bigps1", tag="big")
        for n in range(4):
            tgt = bigps[:, n * 512:(n + 1) * 512]
            nsl = slice(n * 512, (n + 1) * 512)
            for kt in range(8):
                nc.tensor.matmul(tgt, lhsT=w1_sb[:, kt, msl],
                                 rhs=y0[kt][:, nsl], start=(kt == 0),
                                 stop=(kt == 7))
        nc.scalar.activation(out=y1[m], in_=bigps, func=ACT.Identity,
                             accum_out=stats1[:, m:m + 1])
        sc = scr.tile([P, BC], bf, name="sc1", tag="sc")
        nc.vector.scalar_tensor_tensor(out=sc, in0=y1[m], scalar=0.0,
                                       in1=y1[m], op0=ALU.add, op1=ALU.mult,
                                       accum_out=stats1[:, 2 + m:3 + m])

    nc.sync.dma_start(out=cc1_in, in_=stats1)
    nc.gpsimd.collective_compute(
        "AllReduce", ALU.add, replica_groups=[list(range(NCORES))],
        ins=[cc1_in.opt()], outs=[cc1_out.opt()])
    gstats1 = spool.tile([P, 4], f32, name="gstats1")
    nc.sync.dma_start(out=gstats1, in_=cc1_out)

    scale1, shift1 = bn_coefs(gstats1, "1", g1_sb, be1_sb, 2)
    nc.scalar.activation(out=y1[0], in_=y1[0], func=ACT.Relu,
                         bias=shift1[:, 0:1], scale=scale1[:, 0:1])
    nc.vector.tensor_scalar(out=y1[1], in0=y1[1], scalar1=scale1[:, 1:2],
                            scalar2=shift1[:, 1:2], op0=ALU.mult, op1=ALU.add)
    nc.vector.tensor_scalar(out=y1[1], in0=y1[1], scalar1=0.0, scalar2=None,
                            op0=ALU.max)

    out_sb = spool.tile([1, BC], f32, name="out_sb")
    for n in range(4):
        nsl = slice(n * 512, (n + 1) * 512)
        psf = ps_big.tile([1, 512], f32, name="psf", tag="big")
        for kt in range(2):
            nc.tensor.matmul(psf, lhsT=fcw_sb[:, kt, :], rhs=y1[kt][:, nsl],
                             start=(kt == 0), stop=(kt == 1))
        nc.scalar.activation(out=out_sb[:, nsl], in_=psf, func=ACT.Identity,
                             bias=fcb_sb[0:1, 0:1])
    nc.sync.dma_start(out=out_d, in_=out_sb)


# ---------------- module build + run -----------------------------------------
_CACHE = {}


def build_module():
    from contextlib import ExitStack
    import concourse.bacc as bacc
    import concourse.tile as tile
    _register_dve_ops()
    nc = bacc.Bacc("TRN2", target_bir_lowering=False, debug=False,
                   num_devices=NCORES)
    with tile.TileContext(nc) as tc:
        with ExitStack() as ctx:
            _emit(nc, tc, ctx)
    nc.compile()
    return nc


def _get_module():
    if "nc" not in _CACHE:
        _CACHE["nc"] = build_module()
    return _CACHE["nc"]


def host_prep(inputs):
    """Build per-core input maps from the full input dict."""
    gi = {k: np.asarray(v) for k, v in inputs.items()}
    P = 128
    x = gi["x"].astype(np.int64)
    t_id = x[:, 0]
    u_id = np.clip(x[:, 1], 0, EMB_ROWS - 1).astype(np.int32)
    i_id = np.clip(x[:, 2] + NUM_USERS, 0, EMB_ROWS - 1).astype(np.int32)

    emb = np.asarray(gi["embedding"], np.float32)
    T = emb[TROW0:TROW0 + NTROWS]                      # (135, 256)
    # host-folded x-parts of the cell (fp32), -> [p, chunk, col] bf16
    G = (T @ np.asarray(gi["Wg"], np.float32)
         + np.asarray(gi["bg"], np.float32)[None, :])  # (135, 768)
    A = np.einsum("cf,kfe->cke", T,
                  np.asarray(gi["sub_Wx"], np.float32)) # (135, 2, 256)
    gpre = np.ascontiguousarray(
        G.T.reshape(6, P, NTROWS).transpose(1, 0, 2)).astype(BF16)
    # A[c, k, e] -> chunks (k, ftchunk): [p, k*2+ft, c]
    apre = np.ascontiguousarray(
        A.transpose(1, 2, 0).reshape(2, 2, P, NTROWS)
        .transpose(2, 0, 1, 3).reshape(P, 4, NTROWS)).astype(BF16)

    def kchunks(w, nk, width):                          # (nk*128, width) -> [p, nk, width]
        return np.ascontiguousarray(
            np.asarray(w, np.float32).reshape(nk, P, width)
            .transpose(1, 0, 2)).astype(BF16)

    ug16 = kchunks(gi["Ug"], 2, 3 * D)
    wh16 = np.ascontiguousarray(
        np.asarray(gi["sub_Wh"], np.float32).reshape(2, 2, P, D)
        .transpose(2, 0, 1, 3)).astype(BF16)            # [p, k, kt, e]
    aggw16 = kchunks(gi["aggW"], 4, D)
    projw16 = kchunks(gi["projW"], 2, D)
    w0_16 = kchunks(gi["W0"], 6, MLP0)
    w1_16 = kchunks(gi["W1"], 8, MLP1)
    fcw16 = kchunks(gi["fcW"], 2, 1)

    def pcol(v, nt):                                    # (nt*128,) -> [p, nt]
        return np.ascontiguousarray(
            np.asarray(v, np.float32).reshape(nt, P).T)

    mix = np.asarray(gi["sub_mix"], np.float32)         # (2, 2, 256)
    mixsb = np.ascontiguousarray(
        mix.reshape(2, 2, 2, P).transpose(3, 0, 1, 2))  # [p, k, j, ft]

    pc = _fit_poly(gi["spline_coef"], CFG["poly_deg"])  # ascending, deg+1
    spco = np.zeros((1, 8), np.float32)
    spco[0, :len(pc)] = pc

    shared = dict(
        emb16=np.ascontiguousarray(emb.astype(BF16)),
        gpre=gpre, apre=apre, ug=ug16, wh=wh16, aggw=aggw16,
        projw=projw16, w0=w0_16, w1=w1_16, fcw=fcw16,
        aggb=pcol(gi["aggb"], 2),
        projbrow=np.ascontiguousarray(
            np.asarray(gi["projb"], np.float32)[None, :]).astype(BF16),
        mixsb=mixsb, spco=spco,
        gamma0=pcol(gi["gamma0"], 8), beta0=pcol(gi["beta0"], 8),
        gamma1=pcol(gi["gamma1"], 2), beta1=pcol(gi["beta1"], 2),
        fcb=np.asarray(gi["fcb"], np.float32).reshape(1, 1),
    )

    in_maps = []
    for c in range(NCORES):
        rows = slice(c * BC, (c + 1) * BC)
        tid_c = t_id[rows]
        onehot = np.zeros((P, BC), BF16)
        onehot[tid_c, np.arange(BC)] = 1.0
        m = dict(shared)
        m["onehot"] = onehot
        m["uidx"] = np.ascontiguousarray(u_id[rows].reshape(NB, P).T)
        m["iidx"] = np.ascontiguousarray(i_id[rows].reshape(NB, P).T)
        in_maps.append(m)
    return in_maps


def kernel(**inputs):
    from concourse import bass_utils
    nc = _get_module()
    in_maps = host_prep(inputs)
    res = bass_utils.run_bass_kernel_spmd(
        nc, in_maps, core_ids=list(range(NCORES)))
    _CACHE["last_results"] = res
    out = np.concatenate([res.results[c]["out"] for c in range(NCORES)])
    return out.astype(np.float32)


# revision 36
# speedup vs baseline: 2.0114x; 1.0304x over previous
"""Trainium2 Bass kernel for nn_NeuralTensorFactorization (8-core SPMD), v2.

Design (validated numerically in proto.py, rel err ~5.5e-3 vs fp64, tol 2e-2):
- Scan over the 128 distinct tids (batch-independent), feature-major, bf16
  matmuls (4x over f32), fp32 psum/c-state. x-dependent parts of the cell
  (x@Wg+bg, x@Wx) are host-folded into G (128,6,135) / A (128,4,135) tables
  (linear weight folding over the 135 embedding time rows); the device scan
  keeps the recurrence: Ug*h, Wh*sub, spline, mix, aggW, LSTM cell.
- Spline -> degree-6 poly (host minimax fit), evaluated with custom fused DVE
  ops (CLAMPSUM chain) ~5 DVE ops vs ~21 stock.
- Batch 16384 sharded 2048/core; u/i embeddings gathered from a bf16 copy of
  the table, PE-transposed to feature-major bf16 zk tiles.
- MLP0 via 5 matmuls/chunk: 4 emb chunks + V@onehot where V = tfc^T @ W0[512:]
  (associativity; replaces separate one-hot gather of t_feat).
- BN: per-feature (=partition) sum/sumsq accumulated on psum->sbuf copy (Act)
  and square (DVE), two tiny AllReduces; t_feat centered by tid-mean to avoid
  fp32 variance cancellation. bf16 y0/y1 storage.
"""

import numpy as np
import ml_dtypes

# ---------------- model constants (hardcoded; must match the problem) --------
NUM_TIMES, NUM_USERS, NUM_ITEMS = 128, 5000, 50000
STEP, D, B = 8, 256, 16384
MLP0, MLP1 = 1024, 256
NCORES = 8
BC = B // NCORES              # 2048 rows per core
NB = BC // 128                # 16 batch chunks of 128
TROW0 = 55000                 # first embedding row used by the scan windows
NTROWS = NUM_TIMES + STEP - 1 # 135 rows
EMB_ROWS = NUM_TIMES + NUM_USERS + NUM_ITEMS + STEP  # 55136

CFG = dict(
    probes=False,
    poly_deg=6,
    batched_gather=False,   # one indirect DMA per source vs per-128-row chunk
    seed_mode="mm3d",       # "mm3d" | "mm2d" | "dve": how G/A enter psum
)

BF16 = ml_dtypes.bfloat16

# ---------------- host-side spline poly fit (fp64) ---------------------------
SPLINE_ORDER, GRID_SIZE = 3, 5
_H = 2.0 / GRID_SIZE
_KNOTS = np.arange(-SPLINE_ORDER, GRID_SIZE + SPLINE_ORDER + 1,
                   dtype=np.float64) * _H - 1.0


def _bspline_basis64(x):
    knots = _KNOTS
    xe = np.asarray(x, np.float64)[..., None]
    b = ((xe >= knots[:-1]) & (xe < knots[1:])).astype(np.float64)
    for d in range(1, SPLINE_ORDER + 1):
        left = (xe - knots[:-(d + 1)]) / (knots[d:-1] - knots[:-(d + 1)]) * b[..., :-1]
        right = (knots[d + 1:] - xe) / (knots[d + 1:] - knots[1:-d]) * b[..., 1:]
        b = left + right
    return b


def _fit_poly(coef, deg):
    """Weighted LS fit of the clamped spline on [-1,1]; returns c[k] = coeff
    of t^k, ascending, float32, length deg+1."""
    xs = np.linspace(-1.0, 1.0, 4001)
    ys = _bspline_basis64(xs) @ np.asarray(coef, np.float64)
    w = np.ones_like(xs)
    for b in (-1.0, -0.6, -0.2, 0.2, 0.6, 1.0):
        w += 2.0 * np.exp(-((xs - b) / 0.05) ** 2)
    V = np.vander(xs, deg + 1)            # highest power first
    c = np.linalg.lstsq(V * w[:, None], ys * w, rcond=None)[0]
    return c[::-1].astype(np.float32)     # ascending


# ---------------- custom DVE ops ---------------------------------------------
_DVE_OPS = {}


def _register_dve_ops():
    """Idempotently register the fused DVE ops this kernel uses."""
    if _DVE_OPS:
        return _DVE_OPS
    from concourse import dve_ops as DOPS
    from concourse.dve_spec import (Spec, Src0, Src1, C0, C1, One, Zero,
                                    maxx, minn, sq, lower)
    from concourse.dve_uop import DveOpSpec

    def reg(name, spec):
        if name in DOPS._SUB_OPCODE_FOR_NAME:
            return next(o for o in DOPS.OPS if o.name == name)
        row = DOPS._CUSTOM_DVE_ROW_BASE + len(DOPS.OPS)
        shas = {}
        for v in ("v3", "v4"):
            uops = lower(spec, ver=v)
            shas[v] = DveOpSpec(name=name, opcode=row, uops=uops,
                                rd1_en=DOPS.has_src1(spec)).sha(v)
        op = DOPS.DveOp(name, spec, subdim=False, uops_sha=shas)
        DOPS._SUB_OPCODE_FOR_NAME[name] = row
        DOPS.OPS.append(op)
        DOPS.CUSTOM_DVE_SPECS[name] = spec
        return op

    def _clip(in0, imm2):
        return np.minimum(np.maximum(in0.astype(np.float32), imm2), 1.0)

    # t = clip(in0 + in1, -1, 1)
    _DVE_OPS["CLAMPSUM"] = reg("NTF_CLAMPSUM", Spec(
        body=minn(maxx(Src0 + Src1, Zero - One), One),
        reference=lambda in0, in1, s0, s1, imm2:
            _clip(in0 + in1, -1.0)))
    # out = (in0 + in1)^2
    _DVE_OPS["SUMSQ"] = reg("NTF_SUMSQ", Spec(
        body=sq(Src0 + Src1),
        reference=lambda in0, in1, s0, s1, imm2:
            ((in0.astype(np.float32) + in1) ** 2)))
    # Horner step: out = (in1*in0 + s0)*in0 + s1   (in0 = t, in1 = chain)
    _DVE_OPS["HORN2"] = reg("NTF_HORN2", Spec(
        body=(Src1 * Src0 + C0) * Src0 + C1,
        reference=lambda in0, in1, s0, s1, imm2:
            ((in1 * in0.astype(np.float32) + s0) * in0 + s1)))
    # out = in1*in0 + s0
    _DVE_OPS["HORN1"] = reg("NTF_HORN1", Spec(
        body=Src1 * Src0 + C0,
        reference=lambda in0, in1, s0, s1, imm2:
            (in1 * in0.astype(np.float32) + s0)))
    # out = in0*s0 + in1*s1
    _DVE_OPS["AXPBY"] = reg("NTF_AXPBY", Spec(
        body=Src0 * C0 + Src1 * C1,
        reference=lambda in0, in1, s0, s1, imm2:
            (in0.astype(np.float32) * s0 + in1 * s1)))
    return _DVE_OPS


# ---------------- device program ---------------------------------------------

def _emit(nc, tc, ctx):
    import concourse.bass as bass
    import concourse.mybir as mybir
    from concourse.masks import make_identity
    from contextlib import ExitStack
    OPS = _register_dve_ops()
    dt = mybir.dt
    f32 = dt.float32
    bf = dt.bfloat16
    ACT = mybir.ActivationFunctionType
    ALU = mybir.AluOpType
    P = 128

    # ---- DRAM I/O ----
    def din(name, shape, dtype=f32):
        return nc.dram_tensor(name, shape, dtype, kind="ExternalInput").ap()

    emb16 = din("emb16", (EMB_ROWS, D), bf)
    onehot_d = din("onehot", (P, BC), bf)
    uidx_d = din("uidx", (P, NB), dt.int32)
    iidx_d = din("iidx", (P, NB), dt.int32)
    g_d = din("gpre", (P, 6, NTROWS), bf)        # host-folded Wg^T.x + bg
    a_d = din("apre", (P, 4, NTROWS), bf)        # host-folded Wx^T.x
    ug_d = din("ug", (P, 2, 3 * D), bf)
    wh_d = din("wh", (P, 2, 2, D), bf)
    aggw_d = din("aggw", (P, 4, D), bf)
    projw_d = din("projw", (P, 2, D), bf)
    w0_d = din("w0", (P, 6, MLP0), bf)
    w1_d = din("w1", (P, 8, MLP1), bf)
    fcw_d = din("fcw", (P, 2, 1), bf)
    aggb_d = din("aggbrow", (1, D), bf)
    projb_d = din("projbrow", (1, D), bf)
    mix_d = din("mixsb", (P, 2, 2, 2))
    spco_d = din("spco", (1, 8))
    g0_d = din("gamma0", (P, 8)); be0_d = din("beta0", (P, 8))
    g1_d = din("gamma1", (P, 2)); be1_d = din("beta1", (P, 2))
    fcb_d = din("fcb", (1, 1))

    out_d = nc.dram_tensor("out", (BC,), f32, kind="ExternalOutput").ap()

    ccw_in = nc.dram_tensor("ccw_in", (P, 1), f32).ap()
    ccw_out = nc.dram_tensor("ccw_out", (P, 1), f32, addr_space="Shared").ap()
    cc0_in = nc.dram_tensor("cc0_in", (P, 16), f32).ap()
    cc0_out = nc.dram_tensor("cc0_out", (P, 16), f32, addr_space="Shared").ap()
    cc1_in = nc.dram_tensor("cc1_in", (P, 4), f32).ap()
    cc1_out = nc.dram_tensor("cc1_out", (P, 4), f32, addr_space="Shared").ap()

    # ---- whole-kernel pools ----
    consts = ctx.enter_context(tc.tile_pool(name="consts", bufs=1))
    zpool = ctx.enter_context(tc.tile_pool(name="zpool", bufs=1))

    def load(pool, name, dram_ap, shape, dtype=f32):
        t = pool.tile(shape, dtype, name=name)
        nc.sync.dma_start(out=t, in_=dram_ap)
        return t

    # scan-critical weights first (small; scan can start ~immediately)
    g_sb = load(consts, "g_sb", g_d, [P, 6, NTROWS], bf)
    a_sb = load(consts, "a_sb", a_d, [P, 4, NTROWS], bf)
    ug_sb = load(consts, "ug_sb", ug_d, [P, 2, 3 * D], bf)
    wh_sb = load(consts, "wh_sb", wh_d, [P, 2, 2, D], bf)
    aggw_sb = load(consts, "aggw_sb", aggw_d, [P, 4, D], bf)
    aggb_row = load(consts, "aggb_row", aggb_d, [1, D], bf)
    mix_sb = load(consts, "mix_sb", mix_d, [P, 2, 2, 2])
    spco = load(consts, "spco", spco_d.to_broadcast([P, 8]), [P, 8])
    projw_sb = load(consts, "projw_sb", projw_d, [P, 2, D], bf)
    projb_row = load(consts, "projb_row", projb_d, [1, D], bf)
    uidx_sb = load(consts, "uidx_sb", uidx_d, [P, NB], dt.int32)
    iidx_sb = load(consts, "iidx_sb", iidx_d, [P, NB], dt.int32)
    # big weights stream while the scan runs
    w0_sb = load(consts, "w0_sb", w0_d, [P, 6, MLP0], bf)
    onehot_sb = load(consts, "onehot_sb", onehot_d, [P, BC], bf)
    w1_sb = load(consts, "w1_sb", w1_d, [P, 8, MLP1], bf)
    fcw_sb = load(consts, "fcw_sb", fcw_d, [P, 2, 1], bf)
    g0_sb = load(consts, "g0_sb", g0_d, [P, 8])
    be0_sb = load(consts, "be0_sb", be0_d, [P, 8])
    g1_sb = load(consts, "g1_sb", g1_d, [P, 2])
    be1_sb = load(consts, "be1_sb", be1_d, [P, 2])
    fcb_sb = load(consts, "fcb_sb", fcb_d, [1, 1])

    ident16 = consts.tile([P, P], bf, name="ident16")
    make_identity(nc, ident16)
    ident32 = consts.tile([P, P], f32, name="ident32")
    make_identity(nc, ident32)
    ones_col = consts.tile([P, 1], f32, name="ones_col")
    nc.vector.memset(ones_col, 1.0)
    ones_row = consts.tile([1, P], bf, name="ones_row")
    nc.vector.memset(ones_row, 1.0)

    # collective warmup (absorbs first-collective cold-start)
    warm_sb = consts.tile([P, 1], f32, name="warm_sb")
    nc.vector.memset(warm_sb, 1.0)
    nc.sync.dma_start(out=ccw_in, in_=warm_sb)
    nc.gpsimd.collective_compute(
        "AllReduce", ALU.add, replica_groups=[list(range(NCORES))],
        ins=[ccw_in.opt()], outs=[ccw_out.opt()])

    def sp(j):  # poly coefficient j as per-partition scalar AP
        return spco[:, j:j + 1]

    zk = [zpool.tile([P, BC], bf, name=f"zk{i}") for i in range(4)]
    y0 = [zpool.tile([P, BC], bf, name=f"y0_{m}") for m in range(8)]
    V_sb = zpool.tile([P, MLP0], bf, name="V_sb")

    spool = ctx.enter_context(tc.tile_pool(name="spool", bufs=1))
    stats0 = spool.tile([P, 16], f32, name="stats0")
    stats1 = spool.tile([P, 4], f32, name="stats1")

    # =====================================================================
    # Scan + gathers (scoped pools released before the MLP)
    # =====================================================================
    with ExitStack() as sctx:
        pA = sctx.enter_context(tc.tile_pool(name="pA", bufs=1))
        stp = sctx.enter_context(tc.tile_pool(name="stp", bufs=2))
        ps_tr = sctx.enter_context(tc.tile_pool(name="ps_tr", bufs=2, space="PSUM"))
        scan_ps_scope = ExitStack()
        ps_scan = scan_ps_scope.enter_context(
            tc.tile_pool(name="ps_scan", bufs=1, space="PSUM"))

        # persistent scan state, feature-major. gcell = [g (tanh gate) | c]
        # so one tensor_tensor against ifg computes [i*g | f*c].
        h_T = pA.tile([P, 2, P], bf, name="h_T")
        gcell = pA.tile([P, 4, P], bf, name="gcell")
        sub_T = pA.tile([P, 2, 2, P], bf, name="sub_T")
        nc.gpsimd.memset(gcell[:, 2:4, :], 0.0)
        nc.gpsimd.memset(sub_T, 0.0)

        # ---- gather u/i embedding rows -> [P, NB, D] tiles ----
        gtiles = []
        for src, idx_sb in ((0, uidx_sb), (1, iidx_sb)):
            t = pA.tile([P, NB, D], bf, name=f"gt{src}")
            if CFG["batched_gather"]:
                nc.gpsimd.indirect_dma_start(
                    out=t, out_offset=None, in_=emb16,
                    in_offset=bass.IndirectOffsetOnAxis(ap=idx_sb, axis=0))
            else:
                for c in range(NB):
                    nc.gpsimd.indirect_dma_start(
                        out=t[:, c, :], out_offset=None, in_=emb16,
                        in_offset=bass.IndirectOffsetOnAxis(
                            ap=idx_sb[:, c:c + 1], axis=0))
            gtiles.append(t)

        def gather_block(src, cb):
            # 4 chunks x 2 ft -> two [P,512] psum banks -> 2 copies into zk
            for ft in range(2):
                pt = ps_tr.tile([P, 512], bf, name="pt", tag="pt")
                for j in range(4):
                    c = cb * 4 + j
                    nc.tensor.transpose(out=pt[:, j * P:(j + 1) * P],
                                        in_=gtiles[src][:, c, ft * P:(ft + 1) * P],
                                        identity=ident16)
                dst = zk[2 * src + ft][:, cb * 512:(cb + 1) * 512]
                if (src + ft + cb) % 2 == 0:
                    nc.scalar.copy(out=dst, in_=pt)
                else:
                    nc.vector.tensor_copy(out=dst, in_=pt)

        gather_blocks = [(s, cb) for s in range(2) for cb in range(4)]

        # ---- the scan ----
        GS = lambda m0, m1, s: g_sb[:, m0:m1, s:s + P]
        AS = lambda k0, k1, s: a_sb[:, k0:k1, s:s + P]

        for s in range(STEP):
            first = s == 0
            # gates psum seeded with host-folded x@Wg+bg via identity matmul
            psA = ps_scan.tile([P, 4, P], f32, name="psA", tag="psA", bufs=2)
            psB = ps_scan.tile([P, 2, P], f32, name="psB", tag="psB", bufs=1)
            if CFG["seed_mode"] == "mm3d":
                nc.tensor.matmul(psA, lhsT=ident16, rhs=GS(0, 4, s), start=True,
                                 stop=first, skip_group_check=True)
                nc.tensor.matmul(psB, lhsT=ident16, rhs=GS(4, 6, s), start=True,
                                 stop=first, skip_group_check=True)
            else:
                for m in range(6):
                    tgt = psA[:, m, :] if m < 4 else psB[:, m - 4, :]
                    nc.tensor.matmul(tgt, lhsT=ident16,
                                     rhs=g_sb[:, m, s:s + P], start=True,
                                     stop=first, skip_group_check=True)
            if not first:
                for kt in range(2):
                    for m in range(6):
                        tgt = psA[:, m, :] if m < 4 else psB[:, m - 4, :]
                        nc.tensor.matmul(tgt, lhsT=ug_sb[:, kt, m * P:(m + 1) * P],
                                         rhs=h_T[:, kt, :], start=False,
                                         stop=(kt == 1), skip_group_check=True)
            ifg = stp.tile([P, 4, P], bf, name="ifg", tag="ifg")
            nc.scalar.activation(out=ifg, in_=psA, func=ACT.Sigmoid)
            nc.scalar.activation(out=gcell[:, 0:2, :], in_=psB, func=ACT.Tanh)

            psC = ps_scan.tile([P, 2, 2, P], f32, name="psC", tag="psC",
                               bufs=2)
            if CFG["seed_mode"] == "mm3d":
                nc.tensor.matmul(psC, lhsT=ident16, rhs=AS(0, 4, s), start=True,
                                 stop=first, skip_group_check=True)
            else:
                for k in range(2):
                    for mf in range(2):
                        nc.tensor.matmul(psC[:, k, mf, :], lhsT=ident16,
                                         rhs=a_sb[:, 2 * k + mf, s:s + P],
                                         start=True, stop=first,
                                         skip_group_check=True)
            if not first:
                for k in range(2):
                    for mf in range(2):
                        tgt = psC[:, k, mf, :]
                        for kt in range(2):
                            nc.tensor.matmul(
                                tgt, lhsT=wh_sb[:, k, kt, mf * P:(mf + 1) * P],
                                rhs=sub_T[:, k, kt, :], start=False,
                                stop=(kt == 1), skip_group_check=True)

            sq_ = lambda nm: stp.tile([P, 256], bf, name=nm, tag=nm)
            # spline input t = clip(agg0, -1, 1); s1 = agg1^2
            tcl = sq_("tcl")
            nc.vector.tensor_scalar(out=tcl, in0=psC[:, 0], scalar1=1.0,
                                    scalar2=-1.0, op0=ALU.min, op1=ALU.max)
            s1t = sq_("s1t")
            nc.scalar.activation(out=s1t, in_=psC[:, 1], func=ACT.Square)
            # Horner chain for deg-6 poly (coeffs ascending in spco[0..6])
            h1 = sq_("h1")
            nc.vector.tensor_scalar(out=h1, in0=tcl, scalar1=sp(6),
                                    scalar2=sp(5), op0=ALU.mult, op1=ALU.add)
            h2 = sq_("h2")
            nc.vector._custom_dve(OPS["HORN2"], out=h2, in0=tcl, in1=h1,
                                  s0=sp(4), s1=sp(3))
            h3 = sq_("h3")
            nc.vector._custom_dve(OPS["HORN2"], out=h3, in0=tcl, in1=h2,
                                  s0=sp(2), s1=sp(1))
            s0t = sq_("s0t")
            nc.vector._custom_dve(OPS["HORN1"], out=s0t, in0=tcl, in1=h3,
                                  s0=sp(0))

            # new_sub = mixA*sub_out + mixB*sub  (per (k, ft) partition scalars)
            souts = (s0t, s1t)
            for k in range(2):
                for ft in range(2):
                    nc.vector._custom_dve(
                        OPS["AXPBY"], out=sub_T[:, k, ft, :],
                        in0=souts[k][:, ft * P:(ft + 1) * P],
                        in1=sub_T[:, k, ft, :],
                        s0=mix_sb[:, k, 0, ft:ft + 1],
                        s1=mix_sb[:, k, 1, ft:ft + 1])

            # o = sigmoid(aggW^T [s0;s1] + aggb) -- aggb via rank-1 seed
            psD = ps_scan.tile([P, 2, P], f32, name="psD", tag="psD", bufs=1)
            scat = (s0t[:, 0:P], s0t[:, P:2 * P], s1t[:, 0:P], s1t[:, P:2 * P])
            for m in range(2):
                nc.tensor.matmul(psD[:, m, :], lhsT=aggb_row[0:1, m * P:(m + 1) * P],
                                 rhs=ones_row, start=True, stop=False,
                                 skip_group_check=True)
            for m in range(2):
                tgt = psD[:, m, :]
                for kt in range(4):
                    nc.tensor.matmul(tgt, lhsT=aggw_sb[:, kt, m * P:(m + 1) * P],
                                     rhs=scat[kt], start=False,
                                     stop=(kt == 3), skip_group_check=True)
            o_t = stp.tile([P, 2, P], bf, name="o_t", tag="o_t")
            nc.scalar.activation(out=o_t, in_=psD, func=ACT.Sigmoid)

            # LSTM cell: t12 = [i*g | f*c], c_new = t12[0:2]+t12[2:4], all DVE
            t12 = stp.tile([P, 4, P], bf, name="t12", tag="t12")
            nc.vector.tensor_tensor(out=t12, in0=ifg, in1=gcell, op=ALU.mult)
            nc.vector.tensor_tensor(out=gcell[:, 2:4, :], in0=t12[:, 0:2, :],
                                    in1=t12[:, 2:4, :], op=ALU.add)
            tnc = stp.tile([P, 2, P], bf, name="tnc", tag="tnc")
            nc.scalar.activation(out=tnc, in_=gcell[:, 2:4, :], func=ACT.Tanh)
            nc.vector.tensor_tensor(out=h_T, in0=o_t, in1=tnc, op=ALU.mult)

            # interleave one gather-transpose block per step (fills PE gaps)
            if gather_blocks:
                gather_block(*gather_blocks.pop(0))

        # ---- t_feat (tid-major), tid-mean centering, transpose, V ----
        scan_ps_scope.close()
        ps_v = sctx.enter_context(tc.tile_pool(name="ps_v", bufs=1, space="PSUM"))
        psT = ps_v.tile([P, D], f32, name="psT")
        nc.tensor.matmul(psT, lhsT=ones_row, rhs=projb_row, start=True,
                         stop=False)
        for kt in range(2):
            nc.tensor.matmul(psT, lhsT=h_T[:, kt, :], rhs=projw_sb[:, kt, :],
                             start=False, stop=(kt == 1))
        tf = pA.tile([P, D], f32, name="tf")
        nc.scalar.activation(out=tf, in_=psT, func=ACT.Sigmoid)

        # tid-mean via ones-matmul (contract tid partitions)
        psb = ps_v.tile([P, 2], f32, name="psb")
        for j in range(2):
            nc.tensor.matmul(psb[:, j:j + 1], lhsT=tf[:, j * P:(j + 1) * P],
                             rhs=ones_col, start=True, stop=True)
        tbar = pA.tile([P, 2], f32, name="tbar")
        nc.scalar.mul(out=tbar, in_=psb, mul=1.0 / 128.0)

        # transpose tf -> feature-major, subtract tbar
        tfT = pA.tile([P, 2, P], bf, name="tfT")
        for j in range(2):
            pt2 = ps_v.tile([P, P], f32, name="pt2", tag="pt2", bufs=2)
            nc.tensor.transpose(out=pt2, in_=tf[:, j * P:(j + 1) * P],
                                identity=ident32)
            nc.vector.tensor_scalar(out=tfT[:, j, :], in0=pt2,
                                    scalar1=tbar[:, j:j + 1], scalar2=None,
                                    op0=ALU.subtract)

        # V = tfc^T @ W0[512:768]  -> [128 tid, 1024]
        psV = ps_v.tile([P, MLP0], f32, name="psV")
        for half in range(2):
            tgt = psV[:, half * 512:(half + 1) * 512]
            for kt in range(2):
                nc.tensor.matmul(tgt, lhsT=tfT[:, kt, :],
                                 rhs=w0_sb[:, 4 + kt, half * 512:(half + 1) * 512],
                                 start=(kt == 0), stop=(kt == 1))
        nc.scalar.copy(out=V_sb[:, 0:512], in_=psV[:, 0:512])
        nc.vector.tensor_copy(out=V_sb[:, 512:1024], in_=psV[:, 512:1024])

        # leftover gather blocks
        while gather_blocks:
            gather_block(*gather_blocks.pop(0))

    # =====================================================================
    # MLP0: y0^T = W0^T z  (4 emb chunks + V@onehot), stats on the fly
    # =====================================================================
    scr = ctx.enter_context(tc.tile_pool(name="scr", bufs=2))
    ps_big = ctx.enter_context(tc.tile_pool(name="ps_big", bufs=2, space="PSUM"))

    for m in range(8):
        msl = slice(m * P, (m + 1) * P)
        bigps = ps_big.tile([P, BC], f32, name="bigps", tag="big")
        for n in range(4):
            tgt = bigps[:, n * 512:(n + 1) * 512]
            nsl = slice(n * 512, (n + 1) * 512)
            for i, kt in enumerate((0, 1, 2, 3)):
                nc.tensor.matmul(tgt, lhsT=w0_sb[:, kt, msl],
                                 rhs=zk[kt][:, nsl], start=(i == 0), stop=False)
            nc.tensor.matmul(tgt, lhsT=V_sb[:, msl], rhs=onehot_sb[:, nsl],
                             start=False, stop=True)
        nc.scalar.activation(out=y0[m], in_=bigps, func=ACT.Identity,
                             accum_out=stats0[:, m:m + 1])
        sc = scr.tile([P, BC], bf, name="sc0", tag="sc")
        nc.vector.scalar_tensor_tensor(out=sc, in0=y0[m], scalar=0.0,
                                       in1=y0[m], op0=ALU.add, op1=ALU.mult,
                                       accum_out=stats0[:, 8 + m:9 + m])

    nc.sync.dma_start(out=cc0_in, in_=stats0)
    nc.gpsimd.collective_compute(
        "AllReduce", ALU.add, replica_groups=[list(range(NCORES))],
        ins=[cc0_in.opt()], outs=[cc0_out.opt()])
    gstats0 = spool.tile([P, 16], f32, name="gstats0")
    nc.sync.dma_start(out=gstats0, in_=cc0_out)

    def bn_coefs(gstats, nm, gamma_sb, beta_sb, width):
        def t(name):
            return spool.tile([P, width], f32, name=f"{name}{nm}")
        mu = t("mu")
        nc.vector.tensor_scalar(out=mu, in0=gstats[:, 0:width], scalar1=1.0 / B,
                                scalar2=None, op0=ALU.mult)
        ey2 = t("ey2")
        nc.vector.tensor_scalar(out=ey2, in0=gstats[:, width:2 * width],
                                scalar1=1.0 / B, scalar2=None, op0=ALU.mult)
        var = t("var")
        nc.vector.scalar_tensor_tensor(out=var, in0=mu, scalar=0.0, in1=mu,
                                       op0=ALU.add, op1=ALU.mult)
        nc.vector.tensor_tensor(out=var, in0=ey2, in1=var, op=ALU.subtract)
        vpe = t("vpe")
        nc.vector.tensor_scalar(out=vpe, in0=var, scalar1=1e-5, scalar2=None,
                                op0=ALU.add)
        sd = t("sd")
        nc.scalar.activation(out=sd, in_=vpe, func=ACT.Sqrt)
        rstd = t("rstd")
        nc.vector.reciprocal(out=rstd, in_=sd)
        scale = t("scale")
        nc.vector.tensor_tensor(out=scale, in0=gamma_sb, in1=rstd, op=ALU.mult)
        shift = t("shift")
        nc.vector.tensor_tensor(out=shift, in0=mu, in1=scale, op=ALU.mult)
        nc.vector.tensor_tensor(out=shift, in0=beta_sb, in1=shift,
                                op=ALU.subtract)
        return scale, shift

    scale0, shift0 = bn_coefs(gstats0, "0", g0_sb, be0_sb, 8)
    # bn+relu column-block-wise so MLP1 can start per column block
    for n in range(4):
        nsl = slice(n * 512, (n + 1) * 512)
        for m in range(8):
            if (n + m) % 2 == 0:
                nc.scalar.activation(out=y0[m][:, nsl], in_=y0[m][:, nsl],
                                     func=ACT.Relu, bias=shift0[:, m:m + 1],
                                     scale=scale0[:, m:m + 1])
            else:
                nc.vector.tensor_scalar(out=y0[m][:, nsl], in0=y0[m][:, nsl],
                                        scalar1=scale0[:, m:m + 1],
                                        scalar2=shift0[:, m:m + 1],
                                        op0=ALU.mult, op1=ALU.add)
                nc.vector.tensor_scalar(out=y0[m][:, nsl], in0=y0[m][:, nsl],
                                        scalar1=0.0, scalar2=None, op0=ALU.max)

    # =====================================================================
    # MLP1, stats, AR1, bn+relu, fc, output
    # =====================================================================
    y1 = [zpool.tile([P, BC], bf, name=f"y1_{m}") for m in range(2)]
    for m in range(2):
        msl = slice(m * P, (m + 1) * P)
        bigps = ps_big.tile([P, BC], f32, name="bigps1", tag="big")
        for n in range(4):
            tgt = bigps[:, n * 512:(n + 1) * 512]
            nsl = slice(n * 512, (n + 1) * 512)
            for kt in range(8):
                nc.tensor.matmul(tgt, lhsT=w1_sb[:, kt, msl],
                                 rhs=y0[kt][:, nsl], start=(kt == 0),
                                 stop=(kt == 7))
        nc.scalar.activation(out=y1[m], in_=bigps, func=ACT.Identity,
                             accum_out=stats1[:, m:m + 1])
        sc = scr.tile([P, BC], bf, name="sc1", tag="sc")
        nc.vector.scalar_tensor_tensor(out=sc, in0=y1[m], scalar=0.0,
                                       in1=y1[m], op0=ALU.add, op1=ALU.mult,
                                       accum_out=stats1[:, 2 + m:3 + m])

    nc.sync.dma_start(out=cc1_in, in_=stats1)
    nc.gpsimd.collective_compute(
        "AllReduce", ALU.add, replica_groups=[list(range(NCORES))],
        ins=[cc1_in.opt()], outs=[cc1_out.opt()])
    gstats1 = spool.tile([P, 4], f32, name="gstats1")
    nc.sync.dma_start(out=gstats1, in_=cc1_out)

    scale1, shift1 = bn_coefs(gstats1, "1", g1_sb, be1_sb, 2)
    nc.scalar.activation(out=y1[0], in_=y1[0], func=ACT.Relu,
                         bias=shift1[:, 0:1], scale=scale1[:, 0:1])
    nc.vector.tensor_scalar(out=y1[1], in0=y1[1], scalar1=scale1[:, 1:2],
                            scalar2=shift1[:, 1:2], op0=ALU.mult, op1=ALU.add)
    nc.vector.tensor_scalar(out=y1[1], in0=y1[1], scalar1=0.0, scalar2=None,
                            op0=ALU.max)

    out_sb = spool.tile([1, BC], f32, name="out_sb")
    for n in range(4):
        nsl = slice(n * 512, (n + 1) * 512)
        psf = ps_big.tile([1, 512], f32, name="psf", tag="big")
        for kt in range(2):
            nc.tensor.matmul(psf, lhsT=fcw_sb[:, kt, :], rhs=y1[kt][:, nsl],
                             start=(kt == 0), stop=(kt == 1))
        nc.scalar.activation(out=out_sb[:, nsl], in_=psf, func=ACT.Identity,
                             bias=fcb_sb[0:1, 0:1])
    nc.sync.dma_start(out=out_d, in_=out_sb)


# ---------------- module build + run -----------------------------------------
_CACHE = {}


def build_module():
    from contextlib import ExitStack
    import concourse.bacc as bacc
    import concourse.tile as tile
    _register_dve_ops()
    nc = bacc.Bacc("TRN2", target_bir_lowering=False, debug=False,
                   num_devices=NCORES)
    with tile.TileContext(nc) as tc:
        with ExitStack() as ctx:
            _emit(nc, tc, ctx)
    nc.compile()
    return nc


def _get_module():
    if "nc" not in _CACHE:
        _CACHE["nc"] = build_module()
    return _CACHE["nc"]


def host_prep(inputs):
    """Build per-core input maps from the full input dict."""
    gi = {k: np.asarray(v) for k, v in inputs.items()}
    P = 128
    x = gi["x"].astype(np.int64)
    t_id = x[:, 0]
    u_id = np.clip(x[:, 1], 0, EMB_ROWS - 1).astype(np.int32)
    i_id = np.clip(x[:, 2] + NUM_USERS, 0, EMB_ROWS - 1).astype(np.int32)

    emb = np.asarray(gi["embedding"], np.float32)
    T = emb[TROW0:TROW0 + NTROWS]                      # (135, 256)
    # host-folded x-parts of the cell (fp32), -> [p, chunk, col] bf16
    G = (T @ np.asarray(gi["Wg"], np.float32)
         + np.asarray(gi["bg"], np.float32)[None, :])  # (135, 768)
    A = np.einsum("cf,kfe->cke", T,
                  np.asarray(gi["sub_Wx"], np.float32)) # (135, 2, 256)
    gpre = np.ascontiguousarray(
        G.T.reshape(6, P, NTROWS).transpose(1, 0, 2)).astype(BF16)
    # A[c, k, e] -> chunks (k, ftchunk): [p, k*2+ft, c]
    apre = np.ascontiguousarray(
        A.transpose(1, 2, 0).reshape(2, 2, P, NTROWS)
        .transpose(2, 0, 1, 3).reshape(P, 4, NTROWS)).astype(BF16)

    def kchunks(w, nk, width):                          # (nk*128, width) -> [p, nk, width]
        return np.ascontiguousarray(
            np.asarray(w, np.float32).reshape(nk, P, width)
            .transpose(1, 0, 2)).astype(BF16)

    ug16 = kchunks(gi["Ug"], 2, 3 * D)
    wh16 = np.ascontiguousarray(
        np.asarray(gi["sub_Wh"], np.float32).reshape(2, 2, P, D)
        .transpose(2, 0, 1, 3)).astype(BF16)            # [p, k, kt, e]
    aggw16 = kchunks(gi["aggW"], 4, D)
    projw16 = kchunks(gi["projW"], 2, D)
    w0_16 = kchunks(gi["W0"], 6, MLP0)
    w1_16 = kchunks(gi["W1"], 8, MLP1)
    fcw16 = kchunks(gi["fcW"], 2, 1)

    def pcol(v, nt):                                    # (nt*128,) -> [p, nt]
        return np.ascontiguousarray(
            np.asarray(v, np.float32).reshape(nt, P).T)

    mix = np.asarray(gi["sub_mix"], np.float32)         # (2, 2, 256)
    mixsb = np.ascontiguousarray(
        mix.reshape(2, 2, 2, P).transpose(3, 0, 1, 2))  # [p, k, j, ft]

    pc = _fit_poly(gi["spline_coef"], CFG["poly_deg"])  # ascending, deg+1
    spco = np.zeros((1, 8), np.float32)
    spco[0, :len(pc)] = pc

    shared = dict(
        emb16=np.ascontiguousarray(emb.astype(BF16)),
        gpre=gpre, apre=apre, ug=ug16, wh=wh16, aggw=aggw16,
        projw=projw16, w0=w0_16, w1=w1_16, fcw=fcw16,
        aggbrow=np.ascontiguousarray(
            np.asarray(gi["aggb"], np.float32)[None, :]).astype(BF16),
        projbrow=np.ascontiguousarray(
            np.asarray(gi["projb"], np.float32)[None, :]).astype(BF16),
        mixsb=mixsb, spco=spco,
        gamma0=pcol(gi["gamma0"], 8), beta0=pcol(gi["beta0"], 8),
        gamma1=pcol(gi["gamma1"], 2), beta1=pcol(gi["beta1"], 2),
        fcb=np.asarray(gi["fcb"], np.float32).reshape(1, 1),
    )

    in_maps = []
    for c in range(NCORES):
        rows = slice(c * BC, (c + 1) * BC)
        tid_c = t_id[rows]
        onehot = np.zeros((P, BC), BF16)
        onehot[tid_c, np.arange(BC)] = 1.0
        m = dict(shared)
        m["onehot"] = onehot
        m["uidx"] = np.ascontiguousarray(u_id[rows].reshape(NB, P).T)
        m["iidx"] = np.ascontiguousarray(i_id[rows].reshape(NB, P).T)
        in_maps.append(m)
    return in_maps


def kernel(**inputs):
    from concourse import bass_utils
    nc = _get_module()
    in_maps = host_prep(inputs)
    res = bass_utils.run_bass_kernel_spmd(
        nc, in_maps, core_ids=list(range(NCORES)))
    _CACHE["last_results"] = res
    out = np.concatenate([res.results[c]["out"] for c in range(NCORES)])
    return out.astype(np.float32)
